# revision 40
# baseline (speedup 1.0000x reference)
"""Trainium2 Bass kernel for the FlowNet-style correlation module.

out[b, u*21+v, i, j] = sum_c x1[b,c,i,j] * x2pad[b,c,i+u,j+v]
with x1, x2: [4, 128, 128, 128] fp32, pad=10, window 21x21 (441 output channels).

Strategy
--------
Sharding: 8 cores = (batch 4) x (H halves). Each core handles one batch's
64-row slab: x1 slice [C=128, 64, 128] (host-rearranged into 8x8 pixel
blocks) and an x2 slice with halos. The half=1 cores' slabs are FLIPPED
vertically on the host (corr with both operands i-flipped equals the
original with u and i reversed, undone during extraction), so every core's
zero row-halo sits uniformly at padded rows [0,10). Neither the +-10 row
halo nor the +-10 col halo is ever materialized: windows that would straddle
a halo stream only their valid rows/cols, and the Gram entries that would
multiply the zero pad are neither computed nor shipped (the host extraction
knows those outputs are exactly 0).

Per core the correlation is computed as blocked Gram matmuls on the tensor
engine: each 8x8 pixel block of x1 (M=64) is a stationary operand on one
64-column half of the PE array (tile_position=(0,64m)), two blocks per PSUM
tile, each streaming its own (up to) 28x28 x2pad window split into two
row-halves that sit in separate PSUM banks of a 2-bank tile.

Inputs are fp16 single-pass (error budget 2e-2 vs measured ~5e-4 end-to-end;
fp8 variants measured over budget). Gram tiles are copied PSUM->SBUF with
fp32->fp16 conversion (DVE/ACT alternating) and shipped fp16. The j-edge
blocks (window width 18 or 26 instead of 28) get per-member 64-partition
copies and partition-ranged output DMAs into dedicated edge tensors.

Each output pixel's 21x21 window is a per-partition band of its Gram tile; a
per-partition-offset band cannot be expressed by any on-chip access pattern
(and DMA has no PSUM route), so the device ships the full Gram tiles and the
host extracts the band while unsharding. 8x8 blocks trade a little Gram
inflation (784/441 untrimmed vs 672/441 for 4x8) for half the tensor-engine
streaming charge (2 x 784 vs 4 x 672 columns per 128 pixels), which is what
binds once the output ships as fp16.

Per-core traffic: 4.5MB in + 11.5MB Gram out (fp16), ~44.5us of DMA at
~360GB/s, overlapped with a ~40us serial PE span; TimelineSim makespan
48.9us (baseline 93.9us).
"""

import numpy as np

import concourse.mybir as mybir
import concourse.tile as tile
from concourse import bacc
from concourse.bass_utils import run_bass_kernel_spmd

# Problem constants (hardcoded; kernel.py must be self-contained).
B, C, H, W = 4, 128, 128, 128
PAD = 10
WIN = 21  # correlation window side; WIN**2 = 441 output channels
N_CORES = 8
ROWS = H // 2  # 64 output rows per core
X2R = ROWS + PAD  # 74 shipped x2 rows (padded rows [10, 84))

# Pixel blocking: 8x8 blocks (M=64), two blocks per PSUM tile via PE
# column-tiling at tile_position (0,0)/(0,64).
DI, DJ = 8, 8
NR, NS = DI + WIN - 1, DJ + WIN - 1  # 28, 28
NBI, NBJ = ROWS // DI, W // DJ  # 8, 16
NBLK = NBI * NBJ  # 128
NPJ = NBJ // 2  # 8 pairs of j-adjacent blocks per block-row
NPAIR = NBLK // 2  # 64

F32 = mybir.dt.float32
F16 = mybir.dt.float16


def _wj(j0):
    """Valid window col width for a block whose padded window starts at j0."""
    return min(j0 + NS, PAD + W) - max(j0, PAD)


def _tr0(bi):
    """First valid padded window row for block-row bi."""
    return max(DI * bi, PAD)


def _nr(bi):
    """Valid window row count for block-row bi (18 / 26 / 28)."""
    return DI * bi + NR - _tr0(bi)


# Per-m col widths of the two edge pair columns (pj=0 and pj=7).
WL0, WL1 = _wj(0), _wj(DJ)  # 18, 26
WR0, WR1 = _wj(W - 2 * DJ), _wj(W - DJ)  # 26, 18
_EDGE_W = {0: (WL0, WL1), NPJ - 1: (WR0, WR1)}

_NC_CACHE = {}

# Tunables (overridable via _build_nc kwargs for experiments).
GRAM_BUFS = 8
PSUM_BUFS = 4  # [128,1024] fp32 tiles = 2 banks each; 4 bufs = all 8 banks
DVE_MOD = 2  # normal pair copied by DVE iff pair % DVE_MOD == 0, else ACT
# Output DMA schedule: entries are either a pair count (contiguous) or an
# explicit pair list; must cover all NPAIR pairs. Each batch may contain at
# most one edge pair (pj 0 or 7). The tail ends with the lone edge pair so
# the post-copy drain is two small partition-ranged transfers.
QSCHED = [4] * 15 + [[60, 61, 62], [63]]
# Input chunk schedule: list of ("x1", blk_lo, blk_hi) / ("x2", row_lo, row_hi)
# (x2 rows in shipped [0,74) coords) in issue order. First chunks are small
# so the PE starts early.
IN_SCHED = [
    ("x1", 0, 16),
    ("x2", 0, 9),
    ("x2", 9, 18),
    ("x2", 18, 34),
    ("x1", 16, 64),
    ("x2", 34, 58),
    ("x1", 64, 128),
    ("x2", 58, 74),
]
# PE clock-ramp warmup: x1-only matmuls into a scratch PSUM tile while the
# first x2 chunk is still in flight (the p-state model charges matmuls 2x
# until the engine has been busy 3us).
WARMUPS = 6


def _batches(qsched):
    """qsched -> list of (normal_pairs, edge_pairs) per DMA batch."""
    batches = []
    q0 = 0
    seen = []
    for qb in qsched:
        pairs = qb if isinstance(qb, list) else list(range(q0, q0 + qb))
        q0 = pairs[-1] + 1 if isinstance(qb, list) else q0 + qb
        seen += pairs
        normals = [p for p in pairs if p % NPJ not in (0, NPJ - 1)]
        edges = [p for p in pairs if p % NPJ in (0, NPJ - 1)]
        assert len(edges) <= 1, "at most one edge pair per DMA batch"
        batches.append((normals, edges))
    assert sorted(seen) == list(range(NPAIR))
    return batches


# Shipping layout for the DEFAULT schedule (extraction must match kernel()).
def _pair_width(pair):
    bi = pair // NPJ
    return 2 * (_nr(bi) // 2) * NS


_NORM_OFF = {}
_off = 0
for _ns_, _es_ in _batches(QSCHED):
    for _p in _ns_:
        _NORM_OFF[_p] = _off
        _off += _pair_width(_p)
GOUT_COLS = _off
# Edge tensors are indexed [64, sum over bi of 2*hr(bi)*w] in bi order.
_EDGE_OFF = {}
for _w in (WL0, WL1, WR0, WR1):
    offs, o = [], 0
    for _bi in range(NBI):
        offs.append(o)
        o += 2 * (_nr(_bi) // 2) * _w
    _EDGE_OFF[_w] = (offs, o)


def _build_nc(
    qsched=None, in_sched=None, gram_bufs=None, psum_bufs=None, dve_mod=None,
    warmups=None,
):
    gram_bufs = GRAM_BUFS if gram_bufs is None else gram_bufs
    psum_bufs = PSUM_BUFS if psum_bufs is None else psum_bufs
    qsched = list(QSCHED) if qsched is None else list(qsched)
    in_sched = list(IN_SCHED) if in_sched is None else list(in_sched)
    dve_mod = DVE_MOD if dve_mod is None else dve_mod
    warmups = WARMUPS if warmups is None else warmups
    key = (
        tuple(tuple(q) if isinstance(q, list) else q for q in qsched),
        tuple(map(tuple, in_sched)), gram_bufs, psum_bufs, dve_mod, warmups,
    )
    if key in _NC_CACHE:
        return _NC_CACHE[key]
    nc = bacc.Bacc("TRN2", target_bir_lowering=False, debug=False, num_devices=N_CORES)
    # x1 arrives host-rearranged so each 8x8 block's 64 pixels are contiguous
    # (the matmul stationary operand AP must have a single free dimension).
    x1d = nc.dram_tensor("x1", [C, NBLK, DI * DJ], F16, kind="ExternalInput")
    # Only valid rows/cols ship (contiguous SBUF destination keeps the
    # transfer above the 512B full-bandwidth knee).
    x2d = nc.dram_tensor("x2", [C, X2R, W], F16, kind="ExternalInput")
    # Normal pairs (pj 1..6), in shipping order (see _NORM_OFF).
    gout = nc.dram_tensor("gout", [128, GOUT_COLS], F16, kind="ExternalOutput")
    # Edge blocks: partition-ranged tensors, one per block column position.
    edram = {
        "goutL0": (nc.dram_tensor("goutL0", [64, _EDGE_OFF[WL0][1]], F16,
                                  kind="ExternalOutput"), WL0),
        "goutL1": (nc.dram_tensor("goutL1", [64, _EDGE_OFF[WL1][1]], F16,
                                  kind="ExternalOutput"), WL1),
        "goutR0": (nc.dram_tensor("goutR0", [64, _EDGE_OFF[WR0][1]], F16,
                                  kind="ExternalOutput"), WR0),
        "goutR1": (nc.dram_tensor("goutR1", [64, _EDGE_OFF[WR1][1]], F16,
                                  kind="ExternalOutput"), WR1),
    }

    batches = _batches(qsched)

    with tile.TileContext(nc) as tc:
        with (
            tc.tile_pool(name="inp", bufs=1) as inp,
            tc.tile_pool(name="gram", bufs=gram_bufs) as gp,
            tc.tile_pool(name="psum", bufs=psum_bufs, space="PSUM") as pp,
        ):
            x1t = inp.tile([C, NBLK, DI * DJ], F16)
            x2t = inp.tile([C, X2R, W], F16)
            wsrc = None
            if warmups:
                # PE clock warmup source: a tile that only the Pool memset
                # writes, so the warmup matmuls depend on nothing else and
                # ramp the engine to full clock before the first real
                # operand lands. The warmups write into the FIRST pair's
                # PSUM tile (emitted in the pair loop below): a separate
                # scratch PSUM tile would overcommit the 8-bank PSUM pool.
                wsrc = inp.tile([C, 8, DI * DJ], F16)
                nc.gpsimd.memset(wsrc[:], 1)
            for entry in in_sched:
                kind, lo, hi = entry[:3]
                if kind == "x1":
                    nc.sync.dma_start(x1t[:, lo:hi, :], x1d[:, lo:hi, :])
                else:
                    nc.sync.dma_start(x2t[:, lo:hi, :], x2d[:, lo:hi, :])

            nout = 0  # running col offset into gout (normal pairs)
            for normals, edges in batches:
                nwidth = sum(_pair_width(p) for p in normals)
                ewide = 0
                if edges:
                    ebi = edges[0] // NPJ
                    ewide = 2 * (_nr(ebi) // 2) * max(WL1, WR0)
                g = gp.tile([128, nwidth + ewide], F16, tag="g")
                for pairs, is_edge in ((normals, False), (edges, True)):
                    goff = 0
                    for pair in pairs:
                        bi, pj = divmod(pair, NPJ)
                        hr = _nr(bi) // 2
                        rb = _tr0(bi) - PAD  # x2t row of the window start
                        ps = pp.tile([128, 1024], F32, tag="ps")
                        if wsrc is not None:
                            # Warmups go into this first tile's banks; the
                            # real matmuls below reset (start=True) every
                            # region the copy later reads.
                            for _ in range(warmups):
                                nc.tensor.matmul(
                                    ps[0:64, 0:512], wsrc[:, 0, :], wsrc[:],
                                    start=True, stop=True,
                                    tile_position=(0, 0),
                                    skip_group_check=True,
                                )
                            wsrc = None
                        ws = []
                        for m in range(2):
                            j0 = (pj * 2 + m) * DJ  # padded-coord window col
                            w = _wj(j0)
                            ws.append(w)
                            clo = max(j0 - PAD, 0)
                            blk = bi * NBJ + pj * 2 + m
                            for h in range(2):
                                rlo = rb + hr * h
                                nc.tensor.matmul(
                                    ps[64 * m : 64 * m + 64,
                                       512 * h : 512 * h + hr * w],
                                    x1t[:, blk, :],
                                    x2t[:, rlo : rlo + hr, clo : clo + w],
                                    start=True, stop=True,
                                    tile_position=(0, 64 * m),
                                    skip_group_check=True,
                                )
                        if not is_edge:
                            # Strided fp32->fp16 copies spanning both banks.
                            wp = 2 * hr * NS
                            src = ps.rearrange("p (two x) -> p two x", two=2)[
                                :, :, 0 : hr * NS
                            ]
                            dst = g[:, goff : goff + wp].rearrange(
                                "p (two n) -> p two n", two=2
                            )
                            if dve_mod == 0:
                                # Split each copy: one bank half per engine
                                # (halves the per-pair copy latency).
                                nc.vector.tensor_copy(dst[:, 0], src[:, 0])
                                nc.scalar.copy(dst[:, 1], src[:, 1])
                            elif pair % dve_mod == 0:
                                nc.vector.tensor_copy(dst, src)
                            else:
                                nc.scalar.copy(dst, src)
                            goff += wp
                        else:
                            # Per-member 64-partition copies (widths differ).
                            for m in range(2):
                                nc2 = hr * ws[m]
                                src = ps[64 * m : 64 * m + 64, :].rearrange(
                                    "p (two x) -> p two x", two=2
                                )[:, :, 0:nc2]
                                dst = g[
                                    64 * m : 64 * m + 64,
                                    nwidth : nwidth + 2 * nc2,
                                ].rearrange("p (two n) -> p two n", two=2)
                                if m == 0:
                                    nc.vector.tensor_copy(dst, src)
                                else:
                                    nc.scalar.copy(dst, src)
                    # Ship this group of the batch.
                    if is_edge:
                        for pair in pairs:
                            bi, pj = divmod(pair, NPJ)
                            hr = _nr(bi) // 2
                            side = "L" if pj == 0 else "R"
                            for m in range(2):
                                dram, w = edram[f"gout{side}{m}"]
                                n2 = 2 * hr * w
                                eo = _EDGE_OFF[w][0][bi]
                                nc.sync.dma_start(
                                    dram[:, eo : eo + n2],
                                    g[64 * m : 64 * m + 64,
                                      nwidth : nwidth + n2],
                                )
                    elif pairs:
                        nc.sync.dma_start(
                            gout[:, nout : nout + nwidth], g[:, 0:nwidth]
                        )
                        nout += nwidth
    nc.compile()
    _NC_CACHE[key] = nc
    return nc


def _shard_inputs(x1, x2):
    """Per-core inputs: core k -> batch k//2, row-half k%2.

    half=1 slabs are flipped vertically (both operands), which maps the
    correlation to the same computation with u and i reversed; the zero
    row-halo then sits at padded rows [0,10) for every core, and only
    padded rows [10,84) ship.
    """
    in_maps = []
    for k in range(N_CORES):
        b, half = k // 2, k % 2
        if half == 0:
            x1s = x1[b, :, 0:ROWS, :]
            x2s = x2[b, :, 0:X2R, :]  # padded rows 10:84 = global 0:74
        else:
            x1s = x1[b, :, ROWS:H, :][:, ::-1, :]
            # flipped padded rows 10:84 = global rows 127..54 descending
            x2s = x2[b, :, H - X2R : H, :][:, ::-1, :]
        x1s = np.ascontiguousarray(
            x1s.reshape(C, NBI, DI, NBJ, DJ)
            .transpose(0, 1, 3, 2, 4)
            .reshape(C, NBLK, DI * DJ)
        ).astype(np.float16)
        x2s = np.ascontiguousarray(x2s).astype(np.float16)
        in_maps.append({"x1": x1s, "x2": x2s})
    return in_maps


# Band-extraction index arrays (built once).  Within a block, partition
# p = il*DJ + jl; a full 28x28 window stores free f = (il+u)*NS + (jl+v).
_IL = np.arange(DI).reshape(DI, 1, 1, 1)
_JL = np.arange(DJ).reshape(1, DJ, 1, 1)
_U = np.arange(WIN).reshape(1, 1, WIN, 1)
_V = np.arange(WIN).reshape(1, 1, 1, WIN)


def _band(arr, nr, w, rshift, cshift):
    """Band-extract blocks whose stored window is row/col-trimmed.

    arr: [..., 64, nr, w]; stored row = il+u+rshift, col = jl+v+cshift;
    out of range means the output is exactly 0 (zero-pad region).
    Returns [..., DI, DJ, WIN, WIN].
    """
    ro = _IL + _U + rshift
    co = _JL + _V + cshift
    valid = (ro >= 0) & (ro < nr) & (co >= 0) & (co < w)
    part = _IL * DJ + _JL
    band = arr[..., part, np.clip(ro, 0, nr - 1), np.clip(co, 0, w - 1)]
    return np.where(valid, band, np.float32(0.0))


def kernel(x1: np.ndarray, x2: np.ndarray) -> np.ndarray:
    x1 = np.asarray(x1, dtype=np.float32)
    x2 = np.asarray(x2, dtype=np.float32)
    nc = _build_nc()
    in_maps = _shard_inputs(x1, x2)
    # Retry once: a freshly-claimed device occasionally reports a transient
    # NRT_EXEC_UNIT_UNRECOVERABLE on the first execution.
    try:
        res = run_bass_kernel_spmd(nc, in_maps, core_ids=list(range(N_CORES)))
    except Exception:
        import time as _time

        _time.sleep(5.0)
        res = run_bass_kernel_spmd(nc, in_maps, core_ids=list(range(N_CORES)))
    out = np.empty((B, WIN * WIN, H, W), dtype=np.float32)
    corr = np.empty((WIN, WIN, ROWS, W), dtype=np.float32)
    for k in range(N_CORES):
        b, half = k // 2, k % 2
        r = res.results[k]
        gnorm = r["gout"].astype(np.float32)
        for bi in range(NBI):
            nr = _nr(bi)
            rshift = DI * bi - _tr0(bi)  # -10 / -2 / 0
            base = _NORM_OFF[bi * NPJ + 1]
            wp = 2 * (nr // 2) * NS
            # [128, 6, nr, NS] -> [6, 2, 64, nr, NS]
            arr = (
                gnorm[:, base : base + 6 * wp]
                .reshape(2, 64, 6, nr, NS)
                .transpose(2, 0, 1, 3, 4)
            )
            band = _band(arr, nr, NS, rshift, 0)  # [6, 2, DI, DJ, WIN, WIN]
            corr[:, :, bi * DI : (bi + 1) * DI, 2 * DJ : W - 2 * DJ] = (
                band.transpose(4, 5, 2, 0, 1, 3).reshape(WIN, WIN, DI, 6 * 2 * DJ)
            )
            for name, w, cshift, jlo in (
                ("goutL0", WL0, -PAD, 0),
                ("goutL1", WL1, DJ - PAD, DJ),
                ("goutR0", WR0, 0, W - 2 * DJ),
                ("goutR1", WR1, 0, W - DJ),
            ):
                eo = _EDGE_OFF[w][0][bi]
                arr = (
                    r[name][:, eo : eo + nr * w]
                    .reshape(64, nr, w)
                    .astype(np.float32)
                )
                band = _band(arr, nr, w, rshift, cshift)  # [DI, DJ, WIN, WIN]
                corr[:, :, bi * DI : (bi + 1) * DI, jlo : jlo + DJ] = (
                    band.transpose(2, 3, 0, 1)
                )
        if half == 0:
            out[b, :, 0:ROWS, :] = corr.reshape(WIN * WIN, ROWS, W)
        else:
            out[b, :, ROWS:H, :] = corr[::-1, :, ::-1, :].reshape(
                WIN * WIN, ROWS, W
            )
    return out


# revision 41
# speedup vs baseline: 1.0003x; 1.0003x over previous
"""Trainium2 Bass kernel for the FlowNet-style correlation module.

out[b, u*21+v, i, j] = sum_c x1[b,c,i,j] * x2pad[b,c,i+u,j+v]
with x1, x2: [4, 128, 128, 128] fp32, pad=10, window 21x21 (441 output channels).

Strategy
--------
Sharding: 8 cores = (batch 4) x (H halves). Each core handles one batch's
64-row slab: x1 slice [C=128, 64, 128] (host-rearranged into 8x8 pixel
blocks) and an x2 slice with halos. The half=1 cores' slabs are FLIPPED
vertically on the host (corr with both operands i-flipped equals the
original with u and i reversed, undone during extraction), so every core's
zero row-halo sits uniformly at padded rows [0,10). Neither the +-10 row
halo nor the +-10 col halo is ever materialized: windows that would straddle
a halo stream only their valid rows/cols, and the Gram entries that would
multiply the zero pad are neither computed nor shipped (the host extraction
knows those outputs are exactly 0).

Per core the correlation is computed as blocked Gram matmuls on the tensor
engine: each 8x8 pixel block of x1 (M=64) is a stationary operand on one
64-column half of the PE array (tile_position=(0,64m)), two blocks per PSUM
tile, each streaming its own (up to) 28x28 x2pad window split into two
row-halves that sit in separate PSUM banks of a 2-bank tile.

Inputs are fp16 single-pass (error budget 2e-2 vs measured ~5e-4 end-to-end;
fp8 variants measured over budget). Gram tiles are copied PSUM->SBUF with
fp32->fp16 conversion (DVE/ACT alternating) and shipped fp16. The j-edge
blocks (window width 18 or 26 instead of 28) get per-member 64-partition
copies and partition-ranged output DMAs into dedicated edge tensors.

Each output pixel's 21x21 window is a per-partition band of its Gram tile; a
per-partition-offset band cannot be expressed by any on-chip access pattern
(and DMA has no PSUM route), so the device ships the full Gram tiles and the
host extracts the band while unsharding. 8x8 blocks trade a little Gram
inflation (784/441 untrimmed vs 672/441 for 4x8) for half the tensor-engine
streaming charge (2 x 784 vs 4 x 672 columns per 128 pixels), which is what
binds once the output ships as fp16.

Per-core traffic: 4.5MB in + 11.5MB Gram out (fp16), ~44.5us of DMA at
~360GB/s, overlapped with a ~40us serial PE span; TimelineSim makespan
48.9us (baseline 93.9us).
"""

import numpy as np

import concourse.mybir as mybir
import concourse.tile as tile
from concourse import bacc
from concourse.bass_utils import run_bass_kernel_spmd

# Problem constants (hardcoded; kernel.py must be self-contained).
B, C, H, W = 4, 128, 128, 128
PAD = 10
WIN = 21  # correlation window side; WIN**2 = 441 output channels
N_CORES = 8
ROWS = H // 2  # 64 output rows per core
X2R = ROWS + PAD  # 74 shipped x2 rows (padded rows [10, 84))

# Pixel blocking: 8x8 blocks (M=64), two blocks per PSUM tile via PE
# column-tiling at tile_position (0,0)/(0,64).
DI, DJ = 8, 8
NR, NS = DI + WIN - 1, DJ + WIN - 1  # 28, 28
NBI, NBJ = ROWS // DI, W // DJ  # 8, 16
NBLK = NBI * NBJ  # 128
NPJ = NBJ // 2  # 8 pairs of j-adjacent blocks per block-row
NPAIR = NBLK // 2  # 64

F32 = mybir.dt.float32
F16 = mybir.dt.float16


def _wj(j0):
    """Valid window col width for a block whose padded window starts at j0."""
    return min(j0 + NS, PAD + W) - max(j0, PAD)


def _tr0(bi):
    """First valid padded window row for block-row bi."""
    return max(DI * bi, PAD)


def _nr(bi):
    """Valid window row count for block-row bi (18 / 26 / 28)."""
    return DI * bi + NR - _tr0(bi)


# Per-m col widths of the two edge pair columns (pj=0 and pj=7).
WL0, WL1 = _wj(0), _wj(DJ)  # 18, 26
WR0, WR1 = _wj(W - 2 * DJ), _wj(W - DJ)  # 26, 18
_EDGE_W = {0: (WL0, WL1), NPJ - 1: (WR0, WR1)}

_NC_CACHE = {}

# Tunables (overridable via _build_nc kwargs for experiments).
GRAM_BUFS = 8
PSUM_BUFS = 4  # [128,1024] fp32 tiles = 2 banks each; 4 bufs = all 8 banks
DVE_MOD = 2  # normal pair copied by DVE iff pair % DVE_MOD == 0, else ACT
# Output DMA schedule: entries are either a pair count (contiguous) or an
# explicit pair list; must cover all NPAIR pairs. Each batch may contain at
# most one edge pair (pj 0 or 7). The tail ends with the lone edge pair so
# the post-copy drain is two small partition-ranged transfers.
QSCHED = [4] * 15 + [[60, 61, 62], [63]]
# Input chunk schedule: list of ("x1", blk_lo, blk_hi) / ("x2", row_lo, row_hi)
# (x2 rows in shipped [0,74) coords) in issue order. First chunks are small
# so the PE starts early.
IN_SCHED = [
    ("x1", 0, 16),
    ("x2", 0, 9),
    ("x2", 9, 18),
    ("x2", 18, 34),
    ("x1", 16, 64),
    ("x2", 34, 58),
    ("x1", 64, 128),
    ("x2", 58, 74),
]
# PE clock-ramp warmup: x1-only matmuls into a scratch PSUM tile while the
# first x2 chunk is still in flight (the p-state model charges matmuls 2x
# until the engine has been busy 3us).
WARMUPS = 6


def _batches(qsched):
    """qsched -> list of (normal_pairs, edge_pairs) per DMA batch."""
    batches = []
    q0 = 0
    seen = []
    for qb in qsched:
        pairs = qb if isinstance(qb, list) else list(range(q0, q0 + qb))
        q0 = pairs[-1] + 1 if isinstance(qb, list) else q0 + qb
        seen += pairs
        normals = [p for p in pairs if p % NPJ not in (0, NPJ - 1)]
        edges = [p for p in pairs if p % NPJ in (0, NPJ - 1)]
        assert len(edges) <= 1, "at most one edge pair per DMA batch"
        batches.append((normals, edges))
    assert sorted(seen) == list(range(NPAIR))
    return batches


# Shipping layout for the DEFAULT schedule (extraction must match kernel()).
def _pair_width(pair):
    bi = pair // NPJ
    return 2 * (_nr(bi) // 2) * NS


_NORM_OFF = {}
_off = 0
for _ns_, _es_ in _batches(QSCHED):
    for _p in _ns_:
        _NORM_OFF[_p] = _off
        _off += _pair_width(_p)
GOUT_COLS = _off
# Edge tensors are indexed [64, sum over bi of 2*hr(bi)*w] in bi order.
_EDGE_OFF = {}
for _w in (WL0, WL1, WR0, WR1):
    offs, o = [], 0
    for _bi in range(NBI):
        offs.append(o)
        o += 2 * (_nr(_bi) // 2) * _w
    _EDGE_OFF[_w] = (offs, o)


def _build_nc(
    qsched=None, in_sched=None, gram_bufs=None, psum_bufs=None, dve_mod=None,
    warmups=None,
):
    gram_bufs = GRAM_BUFS if gram_bufs is None else gram_bufs
    psum_bufs = PSUM_BUFS if psum_bufs is None else psum_bufs
    qsched = list(QSCHED) if qsched is None else list(qsched)
    in_sched = list(IN_SCHED) if in_sched is None else list(in_sched)
    dve_mod = DVE_MOD if dve_mod is None else dve_mod
    warmups = WARMUPS if warmups is None else warmups
    key = (
        tuple(tuple(q) if isinstance(q, list) else q for q in qsched),
        tuple(map(tuple, in_sched)), gram_bufs, psum_bufs, dve_mod, warmups,
    )
    if key in _NC_CACHE:
        return _NC_CACHE[key]
    nc = bacc.Bacc("TRN2", target_bir_lowering=False, debug=False, num_devices=N_CORES)
    # x1 arrives host-rearranged so each 8x8 block's 64 pixels are contiguous
    # (the matmul stationary operand AP must have a single free dimension).
    x1d = nc.dram_tensor("x1", [C, NBLK, DI * DJ], F16, kind="ExternalInput")
    # Only valid rows/cols ship (contiguous SBUF destination keeps the
    # transfer above the 512B full-bandwidth knee).
    x2d = nc.dram_tensor("x2", [C, X2R, W], F16, kind="ExternalInput")
    # Normal pairs (pj 1..6), in shipping order (see _NORM_OFF).
    gout = nc.dram_tensor("gout", [128, GOUT_COLS], F16, kind="ExternalOutput")
    # Edge blocks: partition-ranged tensors, one per block column position.
    edram = {
        "goutL0": (nc.dram_tensor("goutL0", [64, _EDGE_OFF[WL0][1]], F16,
                                  kind="ExternalOutput"), WL0),
        "goutL1": (nc.dram_tensor("goutL1", [64, _EDGE_OFF[WL1][1]], F16,
                                  kind="ExternalOutput"), WL1),
        "goutR0": (nc.dram_tensor("goutR0", [64, _EDGE_OFF[WR0][1]], F16,
                                  kind="ExternalOutput"), WR0),
        "goutR1": (nc.dram_tensor("goutR1", [64, _EDGE_OFF[WR1][1]], F16,
                                  kind="ExternalOutput"), WR1),
    }

    batches = _batches(qsched)

    with tile.TileContext(nc) as tc:
        with (
            tc.tile_pool(name="inp", bufs=1) as inp,
            tc.tile_pool(name="gram", bufs=gram_bufs) as gp,
            tc.tile_pool(name="psum", bufs=psum_bufs, space="PSUM") as pp,
        ):
            x1t = inp.tile([C, NBLK, DI * DJ], F16)
            x2t = inp.tile([C, X2R, W], F16)
            wsrc = None
            if warmups:
                # PE clock warmup source: a tile that only the Pool memset
                # writes, so the warmup matmuls depend on nothing else and
                # ramp the engine to full clock before the first real
                # operand lands. The warmups write into the FIRST pair's
                # PSUM tile (emitted in the pair loop below): a separate
                # scratch PSUM tile would overcommit the 8-bank PSUM pool.
                wsrc = inp.tile([C, 8, DI * DJ], F16)
                nc.gpsimd.memset(wsrc[:], 1)
            for entry in in_sched:
                kind, lo, hi = entry[:3]
                if kind == "x1":
                    nc.sync.dma_start(x1t[:, lo:hi, :], x1d[:, lo:hi, :])
                else:
                    nc.sync.dma_start(x2t[:, lo:hi, :], x2d[:, lo:hi, :])

            nout = 0  # running col offset into gout (normal pairs)
            for normals, edges in batches:
                nwidth = sum(_pair_width(p) for p in normals)
                ewide = 0
                if edges:
                    ebi = edges[0] // NPJ
                    ewide = 2 * (_nr(ebi) // 2) * max(WL1, WR0)
                g = gp.tile([128, nwidth + ewide], F16, tag="g")
                for pairs, is_edge in ((normals, False), (edges, True)):
                    goff = 0
                    for pair in pairs:
                        bi, pj = divmod(pair, NPJ)
                        hr = _nr(bi) // 2
                        rb = _tr0(bi) - PAD  # x2t row of the window start
                        ps = pp.tile([128, 1024], F32, tag="ps")
                        if wsrc is not None:
                            # Warmups go into this first tile's banks; the
                            # real matmuls below reset (start=True) every
                            # region the copy later reads.
                            for _ in range(warmups):
                                nc.tensor.matmul(
                                    ps[0:64, 0:512], wsrc[:, 0, :], wsrc[:],
                                    start=True, stop=True,
                                    tile_position=(0, 0),
                                    skip_group_check=True,
                                )
                            wsrc = None
                        ws = []
                        for m in range(2):
                            j0 = (pj * 2 + m) * DJ  # padded-coord window col
                            w = _wj(j0)
                            ws.append(w)
                            clo = max(j0 - PAD, 0)
                            blk = bi * NBJ + pj * 2 + m
                            for h in range(2):
                                rlo = rb + hr * h
                                nc.tensor.matmul(
                                    ps[64 * m : 64 * m + 64,
                                       512 * h : 512 * h + hr * w],
                                    x1t[:, blk, :],
                                    x2t[:, rlo : rlo + hr, clo : clo + w],
                                    start=True, stop=True,
                                    tile_position=(0, 64 * m),
                                    skip_group_check=True,
                                )
                        if not is_edge:
                            # Strided fp32->fp16 copies spanning both banks.
                            wp = 2 * hr * NS
                            src = ps.rearrange("p (two x) -> p two x", two=2)[
                                :, :, 0 : hr * NS
                            ]
                            dst = g[:, goff : goff + wp].rearrange(
                                "p (two n) -> p two n", two=2
                            )
                            use_dve = pair % dve_mod == 0
                            if pair in (61, 62):
                                # Tail swap: the drain-gating pair 62 goes to
                                # the faster ACT copy (61 takes DVE instead).
                                use_dve = not use_dve
                            if dve_mod == 0:
                                # Split each copy: one bank half per engine
                                # (halves the per-pair copy latency).
                                nc.vector.tensor_copy(dst[:, 0], src[:, 0])
                                nc.scalar.copy(dst[:, 1], src[:, 1])
                            elif use_dve:
                                nc.vector.tensor_copy(dst, src)
                            else:
                                nc.scalar.copy(dst, src)
                            goff += wp
                        else:
                            # Per-member 64-partition copies (widths differ).
                            for m in range(2):
                                nc2 = hr * ws[m]
                                src = ps[64 * m : 64 * m + 64, :].rearrange(
                                    "p (two x) -> p two x", two=2
                                )[:, :, 0:nc2]
                                dst = g[
                                    64 * m : 64 * m + 64,
                                    nwidth : nwidth + 2 * nc2,
                                ].rearrange("p (two n) -> p two n", two=2)
                                if m == 0:
                                    nc.vector.tensor_copy(dst, src)
                                else:
                                    nc.scalar.copy(dst, src)
                    # Ship this group of the batch.
                    if is_edge:
                        for pair in pairs:
                            bi, pj = divmod(pair, NPJ)
                            hr = _nr(bi) // 2
                            side = "L" if pj == 0 else "R"
                            for m in range(2):
                                dram, w = edram[f"gout{side}{m}"]
                                n2 = 2 * hr * w
                                eo = _EDGE_OFF[w][0][bi]
                                nc.sync.dma_start(
                                    dram[:, eo : eo + n2],
                                    g[64 * m : 64 * m + 64,
                                      nwidth : nwidth + n2],
                                )
                    elif pairs:
                        nc.sync.dma_start(
                            gout[:, nout : nout + nwidth], g[:, 0:nwidth]
                        )
                        nout += nwidth
    nc.compile()
    _NC_CACHE[key] = nc
    return nc


def _shard_inputs(x1, x2):
    """Per-core inputs: core k -> batch k//2, row-half k%2.

    half=1 slabs are flipped vertically (both operands), which maps the
    correlation to the same computation with u and i reversed; the zero
    row-halo then sits at padded rows [0,10) for every core, and only
    padded rows [10,84) ship.
    """
    in_maps = []
    for k in range(N_CORES):
        b, half = k // 2, k % 2
        if half == 0:
            x1s = x1[b, :, 0:ROWS, :]
            x2s = x2[b, :, 0:X2R, :]  # padded rows 10:84 = global 0:74
        else:
            x1s = x1[b, :, ROWS:H, :][:, ::-1, :]
            # flipped padded rows 10:84 = global rows 127..54 descending
            x2s = x2[b, :, H - X2R : H, :][:, ::-1, :]
        x1s = np.ascontiguousarray(
            x1s.reshape(C, NBI, DI, NBJ, DJ)
            .transpose(0, 1, 3, 2, 4)
            .reshape(C, NBLK, DI * DJ)
        ).astype(np.float16)
        x2s = np.ascontiguousarray(x2s).astype(np.float16)
        in_maps.append({"x1": x1s, "x2": x2s})
    return in_maps


# Band-extraction index arrays (built once).  Within a block, partition
# p = il*DJ + jl; a full 28x28 window stores free f = (il+u)*NS + (jl+v).
_IL = np.arange(DI).reshape(DI, 1, 1, 1)
_JL = np.arange(DJ).reshape(1, DJ, 1, 1)
_U = np.arange(WIN).reshape(1, 1, WIN, 1)
_V = np.arange(WIN).reshape(1, 1, 1, WIN)


def _band(arr, nr, w, rshift, cshift):
    """Band-extract blocks whose stored window is row/col-trimmed.

    arr: [..., 64, nr, w]; stored row = il+u+rshift, col = jl+v+cshift;
    out of range means the output is exactly 0 (zero-pad region).
    Returns [..., DI, DJ, WIN, WIN].
    """
    ro = _IL + _U + rshift
    co = _JL + _V + cshift
    valid = (ro >= 0) & (ro < nr) & (co >= 0) & (co < w)
    part = _IL * DJ + _JL
    band = arr[..., part, np.clip(ro, 0, nr - 1), np.clip(co, 0, w - 1)]
    return np.where(valid, band, np.float32(0.0))


def kernel(x1: np.ndarray, x2: np.ndarray) -> np.ndarray:
    x1 = np.asarray(x1, dtype=np.float32)
    x2 = np.asarray(x2, dtype=np.float32)
    nc = _build_nc()
    in_maps = _shard_inputs(x1, x2)
    # Retry once: a freshly-claimed device occasionally reports a transient
    # NRT_EXEC_UNIT_UNRECOVERABLE on the first execution.
    try:
        res = run_bass_kernel_spmd(nc, in_maps, core_ids=list(range(N_CORES)))
    except Exception:
        import time as _time

        _time.sleep(5.0)
        res = run_bass_kernel_spmd(nc, in_maps, core_ids=list(range(N_CORES)))
    out = np.empty((B, WIN * WIN, H, W), dtype=np.float32)
    corr = np.empty((WIN, WIN, ROWS, W), dtype=np.float32)
    for k in range(N_CORES):
        b, half = k // 2, k % 2
        r = res.results[k]
        gnorm = r["gout"].astype(np.float32)
        for bi in range(NBI):
            nr = _nr(bi)
            rshift = DI * bi - _tr0(bi)  # -10 / -2 / 0
            base = _NORM_OFF[bi * NPJ + 1]
            wp = 2 * (nr // 2) * NS
            # [128, 6, nr, NS] -> [6, 2, 64, nr, NS]
            arr = (
                gnorm[:, base : base + 6 * wp]
                .reshape(2, 64, 6, nr, NS)
                .transpose(2, 0, 1, 3, 4)
            )
            band = _band(arr, nr, NS, rshift, 0)  # [6, 2, DI, DJ, WIN, WIN]
            corr[:, :, bi * DI : (bi + 1) * DI, 2 * DJ : W - 2 * DJ] = (
                band.transpose(4, 5, 2, 0, 1, 3).reshape(WIN, WIN, DI, 6 * 2 * DJ)
            )
            for name, w, cshift, jlo in (
                ("goutL0", WL0, -PAD, 0),
                ("goutL1", WL1, DJ - PAD, DJ),
                ("goutR0", WR0, 0, W - 2 * DJ),
                ("goutR1", WR1, 0, W - DJ),
            ):
                eo = _EDGE_OFF[w][0][bi]
                arr = (
                    r[name][:, eo : eo + nr * w]
                    .reshape(64, nr, w)
                    .astype(np.float32)
                )
                band = _band(arr, nr, w, rshift, cshift)  # [DI, DJ, WIN, WIN]
                corr[:, :, bi * DI : (bi + 1) * DI, jlo : jlo + DJ] = (
                    band.transpose(2, 3, 0, 1)
                )
        if half == 0:
            out[b, :, 0:ROWS, :] = corr.reshape(WIN * WIN, ROWS, W)
        else:
            out[b, :, ROWS:H, :] = corr[::-1, :, ::-1, :].reshape(
                WIN * WIN, ROWS, W
            )
    return out


# revision 42
# speedup vs baseline: 1.0008x; 1.0005x over previous
"""Trainium2 Bass kernel for the FlowNet-style correlation module.

out[b, u*21+v, i, j] = sum_c x1[b,c,i,j] * x2pad[b,c,i+u,j+v]
with x1, x2: [4, 128, 128, 128] fp32, pad=10, window 21x21 (441 output channels).

Strategy
--------
Sharding: 8 cores = (batch 4) x (H halves). Each core handles one batch's
64-row slab: x1 slice [C=128, 64, 128] (host-rearranged into 8x8 pixel
blocks) and an x2 slice with halos. The half=1 cores' slabs are FLIPPED
vertically on the host (corr with both operands i-flipped equals the
original with u and i reversed, undone during extraction), so every core's
zero row-halo sits uniformly at padded rows [0,10). Neither the +-10 row
halo nor the +-10 col halo is ever materialized: windows that would straddle
a halo stream only their valid rows/cols, and the Gram entries that would
multiply the zero pad are neither computed nor shipped (the host extraction
knows those outputs are exactly 0).

Per core the correlation is computed as blocked Gram matmuls on the tensor
engine: each 8x8 pixel block of x1 (M=64) is a stationary operand on one
64-column half of the PE array (tile_position=(0,64m)), two blocks per PSUM
tile, each streaming its own (up to) 28x28 x2pad window split into two
row-halves that sit in separate PSUM banks of a 2-bank tile.

Inputs are fp16 single-pass (error budget 2e-2 vs measured ~5e-4 end-to-end;
fp8 variants measured over budget). Gram tiles are copied PSUM->SBUF with
fp32->fp16 conversion (DVE/ACT alternating) and shipped fp16. The j-edge
blocks (window width 18 or 26 instead of 28) get per-member 64-partition
copies and partition-ranged output DMAs into dedicated edge tensors.

Each output pixel's 21x21 window is a per-partition band of its Gram tile; a
per-partition-offset band cannot be expressed by any on-chip access pattern
(and DMA has no PSUM route), so the device ships the full Gram tiles and the
host extracts the band while unsharding. 8x8 blocks trade a little Gram
inflation (784/441 untrimmed vs 672/441 for 4x8) for half the tensor-engine
streaming charge (2 x 784 vs 4 x 672 columns per 128 pixels), which is what
binds once the output ships as fp16.

Per-core traffic: 4.5MB in + 11.5MB Gram out (fp16), ~44.5us of DMA at
~360GB/s, overlapped with a ~40us serial PE span; TimelineSim makespan
48.9us (baseline 93.9us).
"""

import numpy as np

import concourse.mybir as mybir
import concourse.tile as tile
from concourse import bacc
from concourse.bass_utils import run_bass_kernel_spmd

# Problem constants (hardcoded; kernel.py must be self-contained).
B, C, H, W = 4, 128, 128, 128
PAD = 10
WIN = 21  # correlation window side; WIN**2 = 441 output channels
N_CORES = 8
ROWS = H // 2  # 64 output rows per core
X2R = ROWS + PAD  # 74 shipped x2 rows (padded rows [10, 84))

# Pixel blocking: 8x8 blocks (M=64), two blocks per PSUM tile via PE
# column-tiling at tile_position (0,0)/(0,64).
DI, DJ = 8, 8
NR, NS = DI + WIN - 1, DJ + WIN - 1  # 28, 28
NBI, NBJ = ROWS // DI, W // DJ  # 8, 16
NBLK = NBI * NBJ  # 128
NPJ = NBJ // 2  # 8 pairs of j-adjacent blocks per block-row
NPAIR = NBLK // 2  # 64

F32 = mybir.dt.float32
F16 = mybir.dt.float16


def _wj(j0):
    """Valid window col width for a block whose padded window starts at j0."""
    return min(j0 + NS, PAD + W) - max(j0, PAD)


def _tr0(bi):
    """First valid padded window row for block-row bi."""
    return max(DI * bi, PAD)


def _nr(bi):
    """Valid window row count for block-row bi (18 / 26 / 28)."""
    return DI * bi + NR - _tr0(bi)


# Per-m col widths of the two edge pair columns (pj=0 and pj=7).
WL0, WL1 = _wj(0), _wj(DJ)  # 18, 26
WR0, WR1 = _wj(W - 2 * DJ), _wj(W - DJ)  # 26, 18
_EDGE_W = {0: (WL0, WL1), NPJ - 1: (WR0, WR1)}

_NC_CACHE = {}

# Tunables (overridable via _build_nc kwargs for experiments).
GRAM_BUFS = 8
PSUM_BUFS = 4  # [128,1024] fp32 tiles = 2 banks each; 4 bufs = all 8 banks
DVE_MOD = 2  # normal pair copied by DVE iff pair % DVE_MOD == 0, else ACT
# Output DMA schedule: entries are either a pair count (contiguous) or an
# explicit pair list; must cover all NPAIR pairs. Each batch may contain at
# most one edge pair (pj 0 or 7). The tail ends with the lone edge pair so
# the post-copy drain is two small partition-ranged transfers.
QSCHED = [4] * 15 + [[60, 61, 62], [63]]
# Input chunk schedule: list of ("x1", blk_lo, blk_hi) / ("x2", row_lo, row_hi)
# (x2 rows in shipped [0,74) coords) in issue order. First chunks are small
# so the PE starts early.
IN_SCHED = [
    ("x1", 0, 16),
    ("x2", 0, 9),
    ("x2", 9, 18),
    ("x2", 18, 34),
    ("x1", 16, 64),
    ("x2", 34, 58),
    ("x1", 64, 128),
    ("x2", 58, 74),
]
# PE clock-ramp warmup: x1-only matmuls into a scratch PSUM tile while the
# first x2 chunk is still in flight (the p-state model charges matmuls 2x
# until the engine has been busy 3us).
WARMUPS = 6


def _batches(qsched):
    """qsched -> list of (normal_pairs, edge_pairs) per DMA batch."""
    batches = []
    q0 = 0
    seen = []
    for qb in qsched:
        pairs = qb if isinstance(qb, list) else list(range(q0, q0 + qb))
        q0 = pairs[-1] + 1 if isinstance(qb, list) else q0 + qb
        seen += pairs
        normals = [p for p in pairs if p % NPJ not in (0, NPJ - 1)]
        edges = [p for p in pairs if p % NPJ in (0, NPJ - 1)]
        assert len(edges) <= 1, "at most one edge pair per DMA batch"
        batches.append((normals, edges))
    assert sorted(seen) == list(range(NPAIR))
    return batches


# Shipping layout for the DEFAULT schedule (extraction must match kernel()).
def _pair_width(pair):
    bi = pair // NPJ
    return 2 * (_nr(bi) // 2) * NS


_NORM_OFF = {}
_off = 0
for _ns_, _es_ in _batches(QSCHED):
    for _p in _ns_:
        _NORM_OFF[_p] = _off
        _off += _pair_width(_p)
GOUT_COLS = _off
# Edge tensors are indexed [64, sum over bi of 2*hr(bi)*w] in bi order.
_EDGE_OFF = {}
for _w in (WL0, WL1, WR0, WR1):
    offs, o = [], 0
    for _bi in range(NBI):
        offs.append(o)
        o += 2 * (_nr(_bi) // 2) * _w
    _EDGE_OFF[_w] = (offs, o)


def _build_nc(
    qsched=None, in_sched=None, gram_bufs=None, psum_bufs=None, dve_mod=None,
    warmups=None,
):
    gram_bufs = GRAM_BUFS if gram_bufs is None else gram_bufs
    psum_bufs = PSUM_BUFS if psum_bufs is None else psum_bufs
    qsched = list(QSCHED) if qsched is None else list(qsched)
    in_sched = list(IN_SCHED) if in_sched is None else list(in_sched)
    dve_mod = DVE_MOD if dve_mod is None else dve_mod
    warmups = WARMUPS if warmups is None else warmups
    key = (
        tuple(tuple(q) if isinstance(q, list) else q for q in qsched),
        tuple(map(tuple, in_sched)), gram_bufs, psum_bufs, dve_mod, warmups,
    )
    if key in _NC_CACHE:
        return _NC_CACHE[key]
    nc = bacc.Bacc("TRN2", target_bir_lowering=False, debug=False, num_devices=N_CORES)
    # x1 arrives host-rearranged so each 8x8 block's 64 pixels are contiguous
    # (the matmul stationary operand AP must have a single free dimension).
    x1d = nc.dram_tensor("x1", [C, NBLK, DI * DJ], F16, kind="ExternalInput")
    # Only valid rows/cols ship (contiguous SBUF destination keeps the
    # transfer above the 512B full-bandwidth knee).
    x2d = nc.dram_tensor("x2", [C, X2R, W], F16, kind="ExternalInput")
    # Normal pairs (pj 1..6), in shipping order (see _NORM_OFF).
    gout = nc.dram_tensor("gout", [128, GOUT_COLS], F16, kind="ExternalOutput")
    # Edge blocks: partition-ranged tensors, one per block column position.
    edram = {
        "goutL0": (nc.dram_tensor("goutL0", [64, _EDGE_OFF[WL0][1]], F16,
                                  kind="ExternalOutput"), WL0),
        "goutL1": (nc.dram_tensor("goutL1", [64, _EDGE_OFF[WL1][1]], F16,
                                  kind="ExternalOutput"), WL1),
        "goutR0": (nc.dram_tensor("goutR0", [64, _EDGE_OFF[WR0][1]], F16,
                                  kind="ExternalOutput"), WR0),
        "goutR1": (nc.dram_tensor("goutR1", [64, _EDGE_OFF[WR1][1]], F16,
                                  kind="ExternalOutput"), WR1),
    }

    batches = _batches(qsched)

    with tile.TileContext(nc) as tc:
        with (
            tc.tile_pool(name="inp", bufs=1) as inp,
            tc.tile_pool(name="gram", bufs=gram_bufs) as gp,
            tc.tile_pool(name="psum", bufs=psum_bufs, space="PSUM") as pp,
        ):
            x1t = inp.tile([C, NBLK, DI * DJ], F16)
            x2t = inp.tile([C, X2R, W], F16)
            wsrc = None
            if warmups:
                # PE clock warmup source: a tile that only the Pool memset
                # writes, so the warmup matmuls depend on nothing else and
                # ramp the engine to full clock before the first real
                # operand lands. The warmups write into the FIRST pair's
                # PSUM tile (emitted in the pair loop below): a separate
                # scratch PSUM tile would overcommit the 8-bank PSUM pool.
                wsrc = inp.tile([C, 8, DI * DJ], F16)
                nc.gpsimd.memset(wsrc[:], 1)
            for entry in in_sched:
                kind, lo, hi = entry[:3]
                if kind == "x1":
                    nc.sync.dma_start(x1t[:, lo:hi, :], x1d[:, lo:hi, :])
                else:
                    nc.sync.dma_start(x2t[:, lo:hi, :], x2d[:, lo:hi, :])

            nout = 0  # running col offset into gout (normal pairs)
            for normals, edges in batches:
                nwidth = sum(_pair_width(p) for p in normals)
                ewide = 0
                if edges:
                    ebi = edges[0] // NPJ
                    ewide = 2 * (_nr(ebi) // 2) * max(WL1, WR0)
                g = gp.tile([128, nwidth + ewide], F16, tag="g")
                for pairs, is_edge in ((normals, False), (edges, True)):
                    goff = 0
                    for pair in pairs:
                        bi, pj = divmod(pair, NPJ)
                        hr = _nr(bi) // 2
                        rb = _tr0(bi) - PAD  # x2t row of the window start
                        ps = pp.tile([128, 1024], F32, tag="ps")
                        if wsrc is not None:
                            # Warmups go into this first tile's banks; the
                            # real matmuls below reset (start=True) every
                            # region the copy later reads.
                            for _ in range(warmups):
                                nc.tensor.matmul(
                                    ps[0:64, 0:512], wsrc[:, 0, :], wsrc[:],
                                    start=True, stop=True,
                                    tile_position=(0, 0),
                                    skip_group_check=True,
                                )
                            wsrc = None
                        ws = [_wj((pj * 2 + m) * DJ) for m in range(2)]
                        # h-outer: both members' h=0 matmuls need only the
                        # first x2 row-chunk, so the PE starts sooner while
                        # the next chunk is still in flight.
                        for h in range(2):
                            for m in range(2):
                                j0 = (pj * 2 + m) * DJ
                                w = ws[m]
                                clo = max(j0 - PAD, 0)
                                blk = bi * NBJ + pj * 2 + m
                                rlo = rb + hr * h
                                nc.tensor.matmul(
                                    ps[64 * m : 64 * m + 64,
                                       512 * h : 512 * h + hr * w],
                                    x1t[:, blk, :],
                                    x2t[:, rlo : rlo + hr, clo : clo + w],
                                    start=True, stop=True,
                                    tile_position=(0, 64 * m),
                                    skip_group_check=True,
                                )
                        if not is_edge:
                            # Strided fp32->fp16 copies spanning both banks.
                            wp = 2 * hr * NS
                            src = ps.rearrange("p (two x) -> p two x", two=2)[
                                :, :, 0 : hr * NS
                            ]
                            dst = g[:, goff : goff + wp].rearrange(
                                "p (two n) -> p two n", two=2
                            )
                            use_dve = pair % dve_mod == 0
                            if pair in (61, 62):
                                # Tail swap: the drain-gating pair 62 goes to
                                # the faster ACT copy (61 takes DVE instead).
                                use_dve = not use_dve
                            if dve_mod == 0:
                                # Split each copy: one bank half per engine
                                # (halves the per-pair copy latency).
                                nc.vector.tensor_copy(dst[:, 0], src[:, 0])
                                nc.scalar.copy(dst[:, 1], src[:, 1])
                            elif use_dve:
                                nc.vector.tensor_copy(dst, src)
                            else:
                                nc.scalar.copy(dst, src)
                            goff += wp
                        else:
                            # Per-member 64-partition copies (widths differ).
                            for m in range(2):
                                nc2 = hr * ws[m]
                                src = ps[64 * m : 64 * m + 64, :].rearrange(
                                    "p (two x) -> p two x", two=2
                                )[:, :, 0:nc2]
                                dst = g[
                                    64 * m : 64 * m + 64,
                                    nwidth : nwidth + 2 * nc2,
                                ].rearrange("p (two n) -> p two n", two=2)
                                if m == 0:
                                    nc.vector.tensor_copy(dst, src)
                                else:
                                    nc.scalar.copy(dst, src)
                    # Ship this group of the batch.
                    if is_edge:
                        for pair in pairs:
                            bi, pj = divmod(pair, NPJ)
                            hr = _nr(bi) // 2
                            side = "L" if pj == 0 else "R"
                            for m in range(2):
                                dram, w = edram[f"gout{side}{m}"]
                                n2 = 2 * hr * w
                                eo = _EDGE_OFF[w][0][bi]
                                nc.sync.dma_start(
                                    dram[:, eo : eo + n2],
                                    g[64 * m : 64 * m + 64,
                                      nwidth : nwidth + n2],
                                )
                    elif pairs:
                        nc.sync.dma_start(
                            gout[:, nout : nout + nwidth], g[:, 0:nwidth]
                        )
                        nout += nwidth
    nc.compile()
    _NC_CACHE[key] = nc
    return nc


def _shard_inputs(x1, x2):
    """Per-core inputs: core k -> batch k//2, row-half k%2.

    half=1 slabs are flipped vertically (both operands), which maps the
    correlation to the same computation with u and i reversed; the zero
    row-halo then sits at padded rows [0,10) for every core, and only
    padded rows [10,84) ship.
    """
    in_maps = []
    for k in range(N_CORES):
        b, half = k // 2, k % 2
        if half == 0:
            x1s = x1[b, :, 0:ROWS, :]
            x2s = x2[b, :, 0:X2R, :]  # padded rows 10:84 = global 0:74
        else:
            x1s = x1[b, :, ROWS:H, :][:, ::-1, :]
            # flipped padded rows 10:84 = global rows 127..54 descending
            x2s = x2[b, :, H - X2R : H, :][:, ::-1, :]
        x1s = np.ascontiguousarray(
            x1s.reshape(C, NBI, DI, NBJ, DJ)
            .transpose(0, 1, 3, 2, 4)
            .reshape(C, NBLK, DI * DJ)
        ).astype(np.float16)
        x2s = np.ascontiguousarray(x2s).astype(np.float16)
        in_maps.append({"x1": x1s, "x2": x2s})
    return in_maps


# Band-extraction index arrays (built once).  Within a block, partition
# p = il*DJ + jl; a full 28x28 window stores free f = (il+u)*NS + (jl+v).
_IL = np.arange(DI).reshape(DI, 1, 1, 1)
_JL = np.arange(DJ).reshape(1, DJ, 1, 1)
_U = np.arange(WIN).reshape(1, 1, WIN, 1)
_V = np.arange(WIN).reshape(1, 1, 1, WIN)


def _band(arr, nr, w, rshift, cshift):
    """Band-extract blocks whose stored window is row/col-trimmed.

    arr: [..., 64, nr, w]; stored row = il+u+rshift, col = jl+v+cshift;
    out of range means the output is exactly 0 (zero-pad region).
    Returns [..., DI, DJ, WIN, WIN].
    """
    ro = _IL + _U + rshift
    co = _JL + _V + cshift
    valid = (ro >= 0) & (ro < nr) & (co >= 0) & (co < w)
    part = _IL * DJ + _JL
    band = arr[..., part, np.clip(ro, 0, nr - 1), np.clip(co, 0, w - 1)]
    return np.where(valid, band, np.float32(0.0))


def kernel(x1: np.ndarray, x2: np.ndarray) -> np.ndarray:
    x1 = np.asarray(x1, dtype=np.float32)
    x2 = np.asarray(x2, dtype=np.float32)
    nc = _build_nc()
    in_maps = _shard_inputs(x1, x2)
    # Retry once: a freshly-claimed device occasionally reports a transient
    # NRT_EXEC_UNIT_UNRECOVERABLE on the first execution.
    try:
        res = run_bass_kernel_spmd(nc, in_maps, core_ids=list(range(N_CORES)))
    except Exception:
        import time as _time

        _time.sleep(5.0)
        res = run_bass_kernel_spmd(nc, in_maps, core_ids=list(range(N_CORES)))
    out = np.empty((B, WIN * WIN, H, W), dtype=np.float32)
    corr = np.empty((WIN, WIN, ROWS, W), dtype=np.float32)
    for k in range(N_CORES):
        b, half = k // 2, k % 2
        r = res.results[k]
        gnorm = r["gout"].astype(np.float32)
        for bi in range(NBI):
            nr = _nr(bi)
            rshift = DI * bi - _tr0(bi)  # -10 / -2 / 0
            base = _NORM_OFF[bi * NPJ + 1]
            wp = 2 * (nr // 2) * NS
            # [128, 6, nr, NS] -> [6, 2, 64, nr, NS]
            arr = (
                gnorm[:, base : base + 6 * wp]
                .reshape(2, 64, 6, nr, NS)
                .transpose(2, 0, 1, 3, 4)
            )
            band = _band(arr, nr, NS, rshift, 0)  # [6, 2, DI, DJ, WIN, WIN]
            corr[:, :, bi * DI : (bi + 1) * DI, 2 * DJ : W - 2 * DJ] = (
                band.transpose(4, 5, 2, 0, 1, 3).reshape(WIN, WIN, DI, 6 * 2 * DJ)
            )
            for name, w, cshift, jlo in (
                ("goutL0", WL0, -PAD, 0),
                ("goutL1", WL1, DJ - PAD, DJ),
                ("goutR0", WR0, 0, W - 2 * DJ),
                ("goutR1", WR1, 0, W - DJ),
            ):
                eo = _EDGE_OFF[w][0][bi]
                arr = (
                    r[name][:, eo : eo + nr * w]
                    .reshape(64, nr, w)
                    .astype(np.float32)
                )
                band = _band(arr, nr, w, rshift, cshift)  # [DI, DJ, WIN, WIN]
                corr[:, :, bi * DI : (bi + 1) * DI, jlo : jlo + DJ] = (
                    band.transpose(2, 3, 0, 1)
                )
        if half == 0:
            out[b, :, 0:ROWS, :] = corr.reshape(WIN * WIN, ROWS, W)
        else:
            out[b, :, ROWS:H, :] = corr[::-1, :, ::-1, :].reshape(
                WIN * WIN, ROWS, W
            )
    return out


# revision 47
# speedup vs baseline: 1.0431x; 1.0423x over previous
"""Trainium2 Bass kernel for the FlowNet-style correlation module.

out[b, u*21+v, i, j] = sum_c x1[b,c,i,j] * x2pad[b,c,i+u,j+v]
with x1, x2: [4, 128, 128, 128] fp32, pad=10, window 21x21 (441 output channels).

Strategy
--------
Sharding: 8 cores = (batch 4) x (H halves). Each core handles one batch's
64-row slab: x1 slice [C=128, 64, 128] (host-rearranged into 8x8 pixel
blocks) and an x2 slice with halos. The half=1 cores' slabs are FLIPPED
vertically on the host (corr with both operands i-flipped equals the
original with u and i reversed, undone during extraction), so every core's
zero row-halo sits uniformly at padded rows [0,10). Neither the +-10 row
halo nor the +-10 col halo is ever materialized: windows that would straddle
a halo stream only their valid rows/cols, and the Gram entries that would
multiply the zero pad are neither computed nor shipped (the host extraction
knows those outputs are exactly 0).

Per core the correlation is computed as blocked Gram matmuls on the tensor
engine: each 8x8 pixel block of x1 (M=64) is a stationary operand on one
64-column half of the PE array (tile_position=(0,64m)), two blocks per PSUM
tile, each streaming its own (up to) 28x28 x2pad window split into two
row-halves that sit in separate PSUM banks of a 2-bank tile.

Inputs are fp16 single-pass (error budget 2e-2 vs measured ~5e-4 end-to-end;
fp8 variants measured over budget). Gram tiles are copied PSUM->SBUF with
fp32->fp16 conversion (DVE/ACT alternating) and shipped fp16. The j-edge
blocks (window width 18 or 26 instead of 28) get per-member 64-partition
copies and partition-ranged output DMAs into dedicated edge tensors.

Each output pixel's 21x21 window is a per-partition band of its Gram tile; a
per-partition-offset band cannot be expressed by any on-chip access pattern
(and DMA has no PSUM route), so the device ships the full Gram tiles and the
host extracts the band while unsharding. 8x8 blocks trade a little Gram
inflation (784/441 untrimmed vs 672/441 for 4x8) for half the tensor-engine
streaming charge (2 x 784 vs 4 x 672 columns per 128 pixels), which is what
binds once the output ships as fp16.

Per-core traffic: 4.5MB in + 11.5MB Gram out (fp16), ~44.5us of DMA at
~360GB/s, overlapped with a ~40us serial PE span; TimelineSim makespan
48.9us (baseline 93.9us).
"""

import ml_dtypes
import numpy as np

import concourse.mybir as mybir
import concourse.tile as tile
from concourse import bacc
from concourse.bass_utils import run_bass_kernel_spmd

# Problem constants (hardcoded; kernel.py must be self-contained).
B, C, H, W = 4, 128, 128, 128
PAD = 10
WIN = 21  # correlation window side; WIN**2 = 441 output channels
N_CORES = 8
ROWS = H // 2  # 64 output rows per core
X2R = ROWS + PAD  # 74 shipped x2 rows (padded rows [10, 84))

# Pixel blocking: 8x8 blocks (M=64), two blocks per PSUM tile via PE
# column-tiling at tile_position (0,0)/(0,64).
DI, DJ = 8, 8
NR, NS = DI + WIN - 1, DJ + WIN - 1  # 28, 28
NBI, NBJ = ROWS // DI, W // DJ  # 8, 16
NBLK = NBI * NBJ  # 128
NPJ = NBJ // 2  # 8 pairs of j-adjacent blocks per block-row
NPAIR = NBLK // 2  # 64

F32 = mybir.dt.float32
F16 = mybir.dt.float16
F8E3 = mybir.dt.float8e3


def _wj(j0):
    """Valid window col width for a block whose padded window starts at j0."""
    return min(j0 + NS, PAD + W) - max(j0, PAD)


def _tr0(bi):
    """First valid padded window row for block-row bi."""
    return max(DI * bi, PAD)


def _nr(bi):
    """Valid window row count for block-row bi (18 / 26 / 28)."""
    return DI * bi + NR - _tr0(bi)


# Per-m col widths of the two edge pair columns (pj=0 and pj=7).
WL0, WL1 = _wj(0), _wj(DJ)  # 18, 26
WR0, WR1 = _wj(W - 2 * DJ), _wj(W - DJ)  # 26, 18
_EDGE_W = {0: (WL0, WL1), NPJ - 1: (WR0, WR1)}

# Pairs computed as one M=128 union-window matmul pair (both blocks as a
# single stationary, streaming the 36-col union): costs ~28% more shipped
# bytes but ~36% less serial PE time; applied to the tail where the PE is
# the critical path. Their (former) edge members ship full-width.
V4SET = frozenset(range(23, 37))


def _wu(pj):
    # Union window width for a V4 pair (both members).
    lo = max(pj * 2 * DJ, PAD)
    hi = min(pj * 2 * DJ + DJ + NS, PAD + W)
    return hi - lo


_NC_CACHE = {}

# Tunables (overridable via _build_nc kwargs for experiments).
GRAM_BUFS = 8
PSUM_BUFS = 4  # [128,1024] fp32 tiles = 2 banks each; 4 bufs = all 8 banks
DVE_MOD = 2  # normal pair copied by DVE iff pair % DVE_MOD == 0, else ACT
# Output DMA schedule: entries are either a pair count (contiguous) or an
# explicit pair list; must cover all NPAIR pairs. Each batch may contain at
# most one edge pair (pj 0 or 7). The tail ends with the lone edge pair so
# the post-copy drain is two small partition-ranged transfers.
QSCHED = [4] * 16
# Input chunk schedule: list of ("x1", blk_lo, blk_hi) / ("x2", row_lo, row_hi)
# (x2 rows in shipped [0,74) coords) in issue order. First chunks are small
# so the PE starts early.
IN_SCHED = [
    ("x1", 0, 16),
    ("x2", 0, 9),
    ("x2", 9, 18),
    ("x2", 18, 34),
    ("x1", 16, 64),
    ("x2", 34, 58),
    ("x1", 64, 128),
    ("x2", 58, 74),
]
# PE clock-ramp warmup: x1-only matmuls into a scratch PSUM tile while the
# first x2 chunk is still in flight (the p-state model charges matmuls 2x
# until the engine has been busy 3us).
WARMUPS = 6


def _batches(qsched):
    """qsched -> list of (normal_pairs, edge_pairs) per DMA batch."""
    batches = []
    q0 = 0
    seen = []
    for qb in qsched:
        pairs = qb if isinstance(qb, list) else list(range(q0, q0 + qb))
        q0 = pairs[-1] + 1 if isinstance(qb, list) else q0 + qb
        seen += pairs
        normals = [p for p in pairs
                   if p % NPJ not in (0, NPJ - 1) or p in V4SET]
        edges = [p for p in pairs
                 if p % NPJ in (0, NPJ - 1) and p not in V4SET]
        assert len(edges) <= 1, "at most one edge pair per DMA batch"
        batches.append((normals, edges))
    assert sorted(seen) == list(range(NPAIR))
    return batches


# Shipping layout for the DEFAULT schedule (extraction must match kernel()).
def _pair_width(pair):
    bi, pj = divmod(pair, NPJ)
    w = _wu(pj) if pair in V4SET else NS
    return 2 * (_nr(bi) // 2) * w


_NORM_OFF = {}
_off = 0
for _ns_, _es_ in _batches(QSCHED):
    for _p in _ns_:
        _NORM_OFF[_p] = _off
        _off += _pair_width(_p)
GOUT_COLS = _off
# Edge tensors are indexed [64, sum over bi of 2*hr(bi)*w] in bi order.
_EDGE_OFF = {}
for _w in (WL0, WL1, WR0, WR1):
    offs, o = [], 0
    for _bi in range(NBI):
        offs.append(o)
        o += 2 * (_nr(_bi) // 2) * _w
    _EDGE_OFF[_w] = (offs, o)


def _build_nc(
    qsched=None, in_sched=None, gram_bufs=None, psum_bufs=None, dve_mod=None,
    warmups=None,
):
    gram_bufs = GRAM_BUFS if gram_bufs is None else gram_bufs
    psum_bufs = PSUM_BUFS if psum_bufs is None else psum_bufs
    qsched = list(QSCHED) if qsched is None else list(qsched)
    in_sched = list(IN_SCHED) if in_sched is None else list(in_sched)
    dve_mod = DVE_MOD if dve_mod is None else dve_mod
    warmups = WARMUPS if warmups is None else warmups
    key = (
        tuple(tuple(q) if isinstance(q, list) else q for q in qsched),
        tuple(map(tuple, in_sched)), gram_bufs, psum_bufs, dve_mod, warmups,
    )
    if key in _NC_CACHE:
        return _NC_CACHE[key]
    nc = bacc.Bacc("TRN2", target_bir_lowering=False, debug=False, num_devices=N_CORES)
    # x1 arrives host-rearranged so each 8x8 block's 64 pixels are contiguous
    # (the matmul stationary operand AP must have a single free dimension).
    x1d = nc.dram_tensor("x1", [C, NBLK, DI * DJ], F16, kind="ExternalInput")
    # Only valid rows/cols ship (contiguous SBUF destination keeps the
    # transfer above the 512B full-bandwidth knee).
    x2d = nc.dram_tensor("x2", [C, X2R, W], F8E3, kind="ExternalInput")
    # Normal pairs (pj 1..6), in shipping order (see _NORM_OFF).
    gout = nc.dram_tensor("gout", [128, GOUT_COLS], F16, kind="ExternalOutput")
    # Edge blocks: partition-ranged tensors, one per block column position.
    edram = {
        "goutL0": (nc.dram_tensor("goutL0", [64, _EDGE_OFF[WL0][1]], F16,
                                  kind="ExternalOutput"), WL0),
        "goutL1": (nc.dram_tensor("goutL1", [64, _EDGE_OFF[WL1][1]], F16,
                                  kind="ExternalOutput"), WL1),
        "goutR0": (nc.dram_tensor("goutR0", [64, _EDGE_OFF[WR0][1]], F16,
                                  kind="ExternalOutput"), WR0),
        "goutR1": (nc.dram_tensor("goutR1", [64, _EDGE_OFF[WR1][1]], F16,
                                  kind="ExternalOutput"), WR1),
    }

    batches = _batches(qsched)

    with tile.TileContext(nc) as tc:
        with (
            tc.tile_pool(name="inp", bufs=1) as inp,
            tc.tile_pool(name="gram", bufs=gram_bufs) as gp,
            tc.tile_pool(name="psum", bufs=psum_bufs, space="PSUM") as pp,
        ):
            x1t = inp.tile([C, NBLK, DI * DJ], F16)
            x2t = inp.tile([C, X2R, W], F8E3)
            wsrc = None
            if warmups:
                # PE clock warmup source: a tile that only the Pool memset
                # writes, so the warmup matmuls depend on nothing else and
                # ramp the engine to full clock before the first real
                # operand lands. The warmups write into the FIRST pair's
                # PSUM tile (emitted in the pair loop below): a separate
                # scratch PSUM tile would overcommit the 8-bank PSUM pool.
                wsrc = inp.tile([C, 8, DI * DJ], F16)
                nc.gpsimd.memset(wsrc[:], 1)
            for entry in in_sched:
                kind, lo, hi = entry[:3]
                if kind == "x1":
                    nc.sync.dma_start(x1t[:, lo:hi, :], x1d[:, lo:hi, :])
                else:
                    nc.sync.dma_start(x2t[:, lo:hi, :], x2d[:, lo:hi, :])

            nout = 0  # running col offset into gout (normal pairs)
            for normals, edges in batches:
                nwidth = sum(_pair_width(p) for p in normals)
                ewide = 0
                if edges:
                    ebi = edges[0] // NPJ
                    ewide = 2 * (_nr(ebi) // 2) * max(WL1, WR0)
                g = gp.tile([128, nwidth + ewide], F16, tag="g")
                for pairs, is_edge in ((normals, False), (edges, True)):
                    goff = 0
                    for pair in pairs:
                        bi, pj = divmod(pair, NPJ)
                        hr = _nr(bi) // 2
                        rb = _tr0(bi) - PAD  # x2t row of the window start
                        ps = pp.tile([128, 1024], F32, tag="ps")
                        if wsrc is not None:
                            # Warmups go into this first tile's banks; the
                            # real matmuls below reset (start=True) every
                            # region the copy later reads.
                            for _ in range(warmups):
                                nc.tensor.matmul(
                                    ps[0:64, 0:512], wsrc[:, 0, :], wsrc[:],
                                    start=True, stop=True,
                                    tile_position=(0, 0),
                                    skip_group_check=True,
                                )
                            wsrc = None
                        if pair in V4SET:
                            # One M=128 matmul per row-half: both blocks as
                            # a single stationary streaming the union window.
                            wU = _wu(pj)
                            clo = max(pj * 2 * DJ - PAD, 0)
                            blk = bi * NBJ + pj * 2
                            for h in range(2):
                                rlo = rb + hr * h
                                nc.tensor.matmul(
                                    ps[:, 512 * h : 512 * h + hr * wU],
                                    x1t[:, blk : blk + 2, :],
                                    x2t[:, rlo : rlo + hr, clo : clo + wU],
                                    start=True, stop=True,
                                    tile_position=(0, 0),
                                    skip_group_check=True,
                                )
                            wp = 2 * hr * wU
                            src = ps.rearrange("p (two x) -> p two x", two=2)[
                                :, :, 0 : hr * wU
                            ]
                            dst = g[:, goff : goff + wp].rearrange(
                                "p (two n) -> p two n", two=2
                            )
                            if pair % dve_mod == 0:
                                nc.vector.tensor_copy(dst, src)
                            else:
                                nc.scalar.copy(dst, src)
                            goff += wp
                            continue
                        ws = [_wj((pj * 2 + m) * DJ) for m in range(2)]
                        # h-outer: both members' h=0 matmuls need only the
                        # first x2 row-chunk, so the PE starts sooner while
                        # the next chunk is still in flight.
                        for h in range(2):
                            for m in range(2):
                                j0 = (pj * 2 + m) * DJ
                                w = ws[m]
                                clo = max(j0 - PAD, 0)
                                blk = bi * NBJ + pj * 2 + m
                                rlo = rb + hr * h
                                nc.tensor.matmul(
                                    ps[64 * m : 64 * m + 64,
                                       512 * h : 512 * h + hr * w],
                                    x1t[:, blk, :],
                                    x2t[:, rlo : rlo + hr, clo : clo + w],
                                    start=True, stop=True,
                                    tile_position=(0, 64 * m),
                                    skip_group_check=True,
                                )
                        if not is_edge:
                            # Strided fp32->fp16 copies spanning both banks.
                            wp = 2 * hr * NS
                            src = ps.rearrange("p (two x) -> p two x", two=2)[
                                :, :, 0 : hr * NS
                            ]
                            dst = g[:, goff : goff + wp].rearrange(
                                "p (two n) -> p two n", two=2
                            )
                            use_dve = pair % dve_mod == 0
                            if pair in (61, 62):
                                # Tail swap: the drain-gating pair 62 goes to
                                # the faster ACT copy (61 takes DVE instead).
                                use_dve = not use_dve
                            if dve_mod == 0:
                                # Split each copy: one bank half per engine
                                # (halves the per-pair copy latency).
                                nc.vector.tensor_copy(dst[:, 0], src[:, 0])
                                nc.scalar.copy(dst[:, 1], src[:, 1])
                            elif use_dve:
                                nc.vector.tensor_copy(dst, src)
                            else:
                                nc.scalar.copy(dst, src)
                            goff += wp
                        else:
                            # Per-member 64-partition copies (widths differ).
                            for m in range(2):
                                nc2 = hr * ws[m]
                                src = ps[64 * m : 64 * m + 64, :].rearrange(
                                    "p (two x) -> p two x", two=2
                                )[:, :, 0:nc2]
                                dst = g[
                                    64 * m : 64 * m + 64,
                                    nwidth : nwidth + 2 * nc2,
                                ].rearrange("p (two n) -> p two n", two=2)
                                if m == 0:
                                    nc.vector.tensor_copy(dst, src)
                                else:
                                    nc.scalar.copy(dst, src)
                    # Ship this group of the batch.
                    if is_edge:
                        for pair in pairs:
                            bi, pj = divmod(pair, NPJ)
                            hr = _nr(bi) // 2
                            side = "L" if pj == 0 else "R"
                            for m in range(2):
                                dram, w = edram[f"gout{side}{m}"]
                                n2 = 2 * hr * w
                                eo = _EDGE_OFF[w][0][bi]
                                nc.sync.dma_start(
                                    dram[:, eo : eo + n2],
                                    g[64 * m : 64 * m + 64,
                                      nwidth : nwidth + n2],
                                )
                    elif pairs:
                        nc.sync.dma_start(
                            gout[:, nout : nout + nwidth], g[:, 0:nwidth]
                        )
                        nout += nwidth
    nc.compile()
    _NC_CACHE[key] = nc
    return nc


def _shard_inputs(x1, x2):
    """Per-core inputs: core k -> batch k//2, row-half k%2.

    half=1 slabs are flipped vertically (both operands), which maps the
    correlation to the same computation with u and i reversed; the zero
    row-halo then sits at padded rows [0,10) for every core, and only
    padded rows [10,84) ship.
    """
    in_maps = []
    for k in range(N_CORES):
        b, half = k // 2, k % 2
        if half == 0:
            x1s = x1[b, :, 0:ROWS, :]
            x2s = x2[b, :, 0:X2R, :]  # padded rows 10:84 = global 0:74
        else:
            x1s = x1[b, :, ROWS:H, :][:, ::-1, :]
            # flipped padded rows 10:84 = global rows 127..54 descending
            x2s = x2[b, :, H - X2R : H, :][:, ::-1, :]
        x1s = np.ascontiguousarray(
            x1s.reshape(C, NBI, DI, NBJ, DJ)
            .transpose(0, 1, 3, 2, 4)
            .reshape(C, NBLK, DI * DJ)
        ).astype(np.float16)
        x2s = np.ascontiguousarray(x2s).astype(ml_dtypes.float8_e3m4)
        in_maps.append({"x1": x1s, "x2": x2s})
    return in_maps


# Band-extraction index arrays (built once).  Within a block, partition
# p = il*DJ + jl; a full 28x28 window stores free f = (il+u)*NS + (jl+v).
_IL = np.arange(DI).reshape(DI, 1, 1, 1)
_JL = np.arange(DJ).reshape(1, DJ, 1, 1)
_U = np.arange(WIN).reshape(1, 1, WIN, 1)
_V = np.arange(WIN).reshape(1, 1, 1, WIN)


def _band(arr, nr, w, rshift, cshift):
    """Band-extract blocks whose stored window is row/col-trimmed.

    arr: [..., 64, nr, w]; stored row = il+u+rshift, col = jl+v+cshift;
    out of range means the output is exactly 0 (zero-pad region).
    Returns [..., DI, DJ, WIN, WIN].
    """
    ro = _IL + _U + rshift
    co = _JL + _V + cshift
    valid = (ro >= 0) & (ro < nr) & (co >= 0) & (co < w)
    part = _IL * DJ + _JL
    band = arr[..., part, np.clip(ro, 0, nr - 1), np.clip(co, 0, w - 1)]
    return np.where(valid, band, np.float32(0.0))


def kernel(x1: np.ndarray, x2: np.ndarray) -> np.ndarray:
    x1 = np.asarray(x1, dtype=np.float32)
    x2 = np.asarray(x2, dtype=np.float32)
    nc = _build_nc()
    in_maps = _shard_inputs(x1, x2)
    # Retry once: a freshly-claimed device occasionally reports a transient
    # NRT_EXEC_UNIT_UNRECOVERABLE on the first execution.
    try:
        res = run_bass_kernel_spmd(nc, in_maps, core_ids=list(range(N_CORES)))
    except Exception:
        import time as _time

        _time.sleep(5.0)
        res = run_bass_kernel_spmd(nc, in_maps, core_ids=list(range(N_CORES)))
    out = np.empty((B, WIN * WIN, H, W), dtype=np.float32)
    corr = np.empty((WIN, WIN, ROWS, W), dtype=np.float32)
    for k in range(N_CORES):
        b, half = k // 2, k % 2
        r = res.results[k]
        gnorm = r["gout"].astype(np.float32)
        for pair in range(NPAIR):
            bi, pj = divmod(pair, NPJ)
            nr = _nr(bi)
            rshift = DI * bi - _tr0(bi)  # -10 / -2 / 0
            if pair in V4SET:
                wU = _wu(pj)
                base = _NORM_OFF[pair]
                arr2 = gnorm[:, base : base + 2 * (nr // 2) * wU].reshape(
                    2, 64, nr, wU
                )
                for m in range(2):
                    cs = m * DJ - (PAD if pj == 0 else 0)
                    band = _band(arr2[m], nr, wU, rshift, cs)
                    corr[:, :, bi * DI : (bi + 1) * DI,
                         (pj * 2 + m) * DJ : (pj * 2 + m + 1) * DJ] = (
                        band.transpose(2, 3, 0, 1)
                    )
            elif pj in (0, NPJ - 1):
                side = "L" if pj == 0 else "R"
                for m in range(2):
                    w = _EDGE_W[pj][m]
                    cs = (m * DJ - PAD) if pj == 0 else 0
                    eo = _EDGE_OFF[w][0][bi]
                    arr = (
                        r[f"gout{side}{m}"][:, eo : eo + nr * w]
                        .reshape(64, nr, w)
                        .astype(np.float32)
                    )
                    band = _band(arr, nr, w, rshift, cs)
                    corr[:, :, bi * DI : (bi + 1) * DI,
                         (pj * 2 + m) * DJ : (pj * 2 + m + 1) * DJ] = (
                        band.transpose(2, 3, 0, 1)
                    )
            else:
                base = _NORM_OFF[pair]
                arr2 = gnorm[:, base : base + 2 * (nr // 2) * NS].reshape(
                    2, 64, nr, NS
                )
                for m in range(2):
                    band = _band(arr2[m], nr, NS, rshift, 0)
                    corr[:, :, bi * DI : (bi + 1) * DI,
                         (pj * 2 + m) * DJ : (pj * 2 + m + 1) * DJ] = (
                        band.transpose(2, 3, 0, 1)
                    )
        if half == 0:
            out[b, :, 0:ROWS, :] = corr.reshape(WIN * WIN, ROWS, W)
        else:
            out[b, :, ROWS:H, :] = corr[::-1, :, ::-1, :].reshape(
                WIN * WIN, ROWS, W
            )
    return out


# revision 48
# speedup vs baseline: 1.0467x; 1.0034x over previous
"""Trainium2 Bass kernel for the FlowNet-style correlation module.

out[b, u*21+v, i, j] = sum_c x1[b,c,i,j] * x2pad[b,c,i+u,j+v]
with x1, x2: [4, 128, 128, 128] fp32, pad=10, window 21x21 (441 output channels).

Strategy
--------
Sharding: 8 cores = (batch 4) x (H halves). Each core handles one batch's
64-row slab: x1 slice [C=128, 64, 128] (host-rearranged into 8x8 pixel
blocks) and an x2 slice with halos. The half=1 cores' slabs are FLIPPED
vertically on the host (corr with both operands i-flipped equals the
original with u and i reversed, undone during extraction), so every core's
zero row-halo sits uniformly at padded rows [0,10). Neither the +-10 row
halo nor the +-10 col halo is ever materialized: windows that would straddle
a halo stream only their valid rows/cols, and the Gram entries that would
multiply the zero pad are neither computed nor shipped (the host extraction
knows those outputs are exactly 0).

Per core the correlation is computed as blocked Gram matmuls on the tensor
engine: each 8x8 pixel block of x1 (M=64) is a stationary operand on one
64-column half of the PE array (tile_position=(0,64m)), two blocks per PSUM
tile, each streaming its own (up to) 28x28 x2pad window split into two
row-halves that sit in separate PSUM banks of a 2-bank tile.

Inputs are fp16 single-pass (error budget 2e-2 vs measured ~5e-4 end-to-end;
fp8 variants measured over budget). Gram tiles are copied PSUM->SBUF with
fp32->fp16 conversion (DVE/ACT alternating) and shipped fp16. The j-edge
blocks (window width 18 or 26 instead of 28) get per-member 64-partition
copies and partition-ranged output DMAs into dedicated edge tensors.

Each output pixel's 21x21 window is a per-partition band of its Gram tile; a
per-partition-offset band cannot be expressed by any on-chip access pattern
(and DMA has no PSUM route), so the device ships the full Gram tiles and the
host extracts the band while unsharding. 8x8 blocks trade a little Gram
inflation (784/441 untrimmed vs 672/441 for 4x8) for half the tensor-engine
streaming charge (2 x 784 vs 4 x 672 columns per 128 pixels), which is what
binds once the output ships as fp16.

Per-core traffic: 4.5MB in + 11.5MB Gram out (fp16), ~44.5us of DMA at
~360GB/s, overlapped with a ~40us serial PE span; TimelineSim makespan
48.9us (baseline 93.9us).
"""

import ml_dtypes
import numpy as np

import concourse.mybir as mybir
import concourse.tile as tile
from concourse import bacc
from concourse.bass_utils import run_bass_kernel_spmd

# Problem constants (hardcoded; kernel.py must be self-contained).
B, C, H, W = 4, 128, 128, 128
PAD = 10
WIN = 21  # correlation window side; WIN**2 = 441 output channels
N_CORES = 8
ROWS = H // 2  # 64 output rows per core
X2R = ROWS + PAD  # 74 shipped x2 rows (padded rows [10, 84))

# Pixel blocking: 8x8 blocks (M=64), two blocks per PSUM tile via PE
# column-tiling at tile_position (0,0)/(0,64).
DI, DJ = 8, 8
NR, NS = DI + WIN - 1, DJ + WIN - 1  # 28, 28
NBI, NBJ = ROWS // DI, W // DJ  # 8, 16
NBLK = NBI * NBJ  # 128
NPJ = NBJ // 2  # 8 pairs of j-adjacent blocks per block-row
NPAIR = NBLK // 2  # 64

F32 = mybir.dt.float32
F16 = mybir.dt.float16
F8E3 = mybir.dt.float8e3


def _wj(j0):
    """Valid window col width for a block whose padded window starts at j0."""
    return min(j0 + NS, PAD + W) - max(j0, PAD)


def _tr0(bi):
    """First valid padded window row for block-row bi."""
    return max(DI * bi, PAD)


def _nr(bi):
    """Valid window row count for block-row bi (18 / 26 / 28)."""
    return DI * bi + NR - _tr0(bi)


# Per-m col widths of the two edge pair columns (pj=0 and pj=7).
WL0, WL1 = _wj(0), _wj(DJ)  # 18, 26
WR0, WR1 = _wj(W - 2 * DJ), _wj(W - DJ)  # 26, 18
_EDGE_W = {0: (WL0, WL1), NPJ - 1: (WR0, WR1)}

# Pairs computed as one M=128 union-window matmul pair (both blocks as a
# single stationary, streaming the 36-col union): costs ~28% more shipped
# bytes but ~36% less serial PE time; applied to the tail where the PE is
# the critical path. Their (former) edge members ship full-width.
V4SET = frozenset(range(23, 37)) - {28}


def _wu(pj):
    # Union window width for a V4 pair (both members).
    lo = max(pj * 2 * DJ, PAD)
    hi = min(pj * 2 * DJ + DJ + NS, PAD + W)
    return hi - lo


_NC_CACHE = {}

# Tunables (overridable via _build_nc kwargs for experiments).
GRAM_BUFS = 8
PSUM_BUFS = 4  # [128,1024] fp32 tiles = 2 banks each; 4 bufs = all 8 banks
DVE_MOD = 2  # normal pair copied by DVE iff pair % DVE_MOD == 0, else ACT
# Output DMA schedule: entries are either a pair count (contiguous) or an
# explicit pair list; must cover all NPAIR pairs. Each batch may contain at
# most one edge pair (pj 0 or 7). The tail ends with the lone edge pair so
# the post-copy drain is two small partition-ranged transfers.
QSCHED = [4] * 16
# Input chunk schedule: list of ("x1", blk_lo, blk_hi) / ("x2", row_lo, row_hi)
# (x2 rows in shipped [0,74) coords) in issue order. First chunks are small
# so the PE starts early.
IN_SCHED = [
    ("x1", 0, 16),
    ("x2", 0, 9),
    ("x2", 9, 18),
    ("x2", 18, 34),
    ("x1", 16, 64),
    ("x2", 34, 58),
    ("x1", 64, 128),
    ("x2", 58, 74),
]
# PE clock-ramp warmup: x1-only matmuls into a scratch PSUM tile while the
# first x2 chunk is still in flight (the p-state model charges matmuls 2x
# until the engine has been busy 3us).
WARMUPS = 6


def _batches(qsched):
    """qsched -> list of (normal_pairs, edge_pairs) per DMA batch."""
    batches = []
    q0 = 0
    seen = []
    for qb in qsched:
        pairs = qb if isinstance(qb, list) else list(range(q0, q0 + qb))
        q0 = pairs[-1] + 1 if isinstance(qb, list) else q0 + qb
        seen += pairs
        normals = [p for p in pairs
                   if p % NPJ not in (0, NPJ - 1) or p in V4SET]
        edges = [p for p in pairs
                 if p % NPJ in (0, NPJ - 1) and p not in V4SET]
        assert len(edges) <= 1, "at most one edge pair per DMA batch"
        batches.append((normals, edges))
    assert sorted(seen) == list(range(NPAIR))
    return batches


# Shipping layout for the DEFAULT schedule (extraction must match kernel()).
def _pair_width(pair):
    bi, pj = divmod(pair, NPJ)
    w = _wu(pj) if pair in V4SET else NS
    return 2 * (_nr(bi) // 2) * w


_NORM_OFF = {}
_off = 0
for _ns_, _es_ in _batches(QSCHED):
    for _p in _ns_:
        _NORM_OFF[_p] = _off
        _off += _pair_width(_p)
GOUT_COLS = _off
# Edge tensors are indexed [64, sum over bi of 2*hr(bi)*w] in bi order.
_EDGE_OFF = {}
for _w in (WL0, WL1, WR0, WR1):
    offs, o = [], 0
    for _bi in range(NBI):
        offs.append(o)
        o += 2 * (_nr(_bi) // 2) * _w
    _EDGE_OFF[_w] = (offs, o)


def _build_nc(
    qsched=None, in_sched=None, gram_bufs=None, psum_bufs=None, dve_mod=None,
    warmups=None,
):
    gram_bufs = GRAM_BUFS if gram_bufs is None else gram_bufs
    psum_bufs = PSUM_BUFS if psum_bufs is None else psum_bufs
    qsched = list(QSCHED) if qsched is None else list(qsched)
    in_sched = list(IN_SCHED) if in_sched is None else list(in_sched)
    dve_mod = DVE_MOD if dve_mod is None else dve_mod
    warmups = WARMUPS if warmups is None else warmups
    key = (
        tuple(tuple(q) if isinstance(q, list) else q for q in qsched),
        tuple(map(tuple, in_sched)), gram_bufs, psum_bufs, dve_mod, warmups,
    )
    if key in _NC_CACHE:
        return _NC_CACHE[key]
    nc = bacc.Bacc("TRN2", target_bir_lowering=False, debug=False, num_devices=N_CORES)
    # x1 arrives host-rearranged so each 8x8 block's 64 pixels are contiguous
    # (the matmul stationary operand AP must have a single free dimension).
    x1d = nc.dram_tensor("x1", [C, NBLK, DI * DJ], F16, kind="ExternalInput")
    # Only valid rows/cols ship (contiguous SBUF destination keeps the
    # transfer above the 512B full-bandwidth knee).
    x2d = nc.dram_tensor("x2", [C, X2R, W], F8E3, kind="ExternalInput")
    # Normal pairs (pj 1..6), in shipping order (see _NORM_OFF).
    gout = nc.dram_tensor("gout", [128, GOUT_COLS], F16, kind="ExternalOutput")
    # Edge blocks: partition-ranged tensors, one per block column position.
    edram = {
        "goutL0": (nc.dram_tensor("goutL0", [64, _EDGE_OFF[WL0][1]], F16,
                                  kind="ExternalOutput"), WL0),
        "goutL1": (nc.dram_tensor("goutL1", [64, _EDGE_OFF[WL1][1]], F16,
                                  kind="ExternalOutput"), WL1),
        "goutR0": (nc.dram_tensor("goutR0", [64, _EDGE_OFF[WR0][1]], F16,
                                  kind="ExternalOutput"), WR0),
        "goutR1": (nc.dram_tensor("goutR1", [64, _EDGE_OFF[WR1][1]], F16,
                                  kind="ExternalOutput"), WR1),
    }

    batches = _batches(qsched)

    with tile.TileContext(nc) as tc:
        with (
            tc.tile_pool(name="inp", bufs=1) as inp,
            tc.tile_pool(name="gram", bufs=gram_bufs) as gp,
            tc.tile_pool(name="psum", bufs=psum_bufs, space="PSUM") as pp,
        ):
            x1t = inp.tile([C, NBLK, DI * DJ], F16)
            x2t = inp.tile([C, X2R, W], F8E3)
            wsrc = None
            if warmups:
                # PE clock warmup source: a tile that only the Pool memset
                # writes, so the warmup matmuls depend on nothing else and
                # ramp the engine to full clock before the first real
                # operand lands. The warmups write into the FIRST pair's
                # PSUM tile (emitted in the pair loop below): a separate
                # scratch PSUM tile would overcommit the 8-bank PSUM pool.
                wsrc = inp.tile([C, 8, DI * DJ], F16)
                nc.gpsimd.memset(wsrc[:], 1)
            for entry in in_sched:
                kind, lo, hi = entry[:3]
                if kind == "x1":
                    nc.sync.dma_start(x1t[:, lo:hi, :], x1d[:, lo:hi, :])
                else:
                    nc.sync.dma_start(x2t[:, lo:hi, :], x2d[:, lo:hi, :])

            nout = 0  # running col offset into gout (normal pairs)
            for normals, edges in batches:
                nwidth = sum(_pair_width(p) for p in normals)
                ewide = 0
                if edges:
                    ebi = edges[0] // NPJ
                    ewide = 2 * (_nr(ebi) // 2) * max(WL1, WR0)
                g = gp.tile([128, nwidth + ewide], F16, tag="g")
                for pairs, is_edge in ((normals, False), (edges, True)):
                    goff = 0
                    for pair in pairs:
                        bi, pj = divmod(pair, NPJ)
                        hr = _nr(bi) // 2
                        rb = _tr0(bi) - PAD  # x2t row of the window start
                        ps = pp.tile([128, 1024], F32, tag="ps")
                        if wsrc is not None:
                            # Warmups go into this first tile's banks; the
                            # real matmuls below reset (start=True) every
                            # region the copy later reads.
                            for _ in range(warmups):
                                nc.tensor.matmul(
                                    ps[0:64, 0:512], wsrc[:, 0, :], wsrc[:],
                                    start=True, stop=True,
                                    tile_position=(0, 0),
                                    skip_group_check=True,
                                )
                            wsrc = None
                        if pair in V4SET:
                            # One M=128 matmul per row-half: both blocks as
                            # a single stationary streaming the union window.
                            wU = _wu(pj)
                            clo = max(pj * 2 * DJ - PAD, 0)
                            blk = bi * NBJ + pj * 2
                            for h in range(2):
                                rlo = rb + hr * h
                                nc.tensor.matmul(
                                    ps[:, 512 * h : 512 * h + hr * wU],
                                    x1t[:, blk : blk + 2, :],
                                    x2t[:, rlo : rlo + hr, clo : clo + wU],
                                    start=True, stop=True,
                                    tile_position=(0, 0),
                                    skip_group_check=True,
                                )
                            wp = 2 * hr * wU
                            src = ps.rearrange("p (two x) -> p two x", two=2)[
                                :, :, 0 : hr * wU
                            ]
                            dst = g[:, goff : goff + wp].rearrange(
                                "p (two n) -> p two n", two=2
                            )
                            if pair % dve_mod == 0:
                                nc.vector.tensor_copy(dst, src)
                            else:
                                nc.scalar.copy(dst, src)
                            goff += wp
                            continue
                        ws = [_wj((pj * 2 + m) * DJ) for m in range(2)]
                        # h-outer: both members' h=0 matmuls need only the
                        # first x2 row-chunk, so the PE starts sooner while
                        # the next chunk is still in flight.
                        for h in range(2):
                            for m in range(2):
                                j0 = (pj * 2 + m) * DJ
                                w = ws[m]
                                clo = max(j0 - PAD, 0)
                                blk = bi * NBJ + pj * 2 + m
                                rlo = rb + hr * h
                                nc.tensor.matmul(
                                    ps[64 * m : 64 * m + 64,
                                       512 * h : 512 * h + hr * w],
                                    x1t[:, blk, :],
                                    x2t[:, rlo : rlo + hr, clo : clo + w],
                                    start=True, stop=True,
                                    tile_position=(0, 64 * m),
                                    skip_group_check=True,
                                )
                        if not is_edge:
                            # Strided fp32->fp16 copies spanning both banks.
                            wp = 2 * hr * NS
                            src = ps.rearrange("p (two x) -> p two x", two=2)[
                                :, :, 0 : hr * NS
                            ]
                            dst = g[:, goff : goff + wp].rearrange(
                                "p (two n) -> p two n", two=2
                            )
                            use_dve = pair % dve_mod == 0
                            if pair in (61, 62):
                                # Tail swap: the drain-gating pair 62 goes to
                                # the faster ACT copy (61 takes DVE instead).
                                use_dve = not use_dve
                            if dve_mod == 0:
                                # Split each copy: one bank half per engine
                                # (halves the per-pair copy latency).
                                nc.vector.tensor_copy(dst[:, 0], src[:, 0])
                                nc.scalar.copy(dst[:, 1], src[:, 1])
                            elif use_dve:
                                nc.vector.tensor_copy(dst, src)
                            else:
                                nc.scalar.copy(dst, src)
                            goff += wp
                        else:
                            # Per-member 64-partition copies (widths differ).
                            for m in range(2):
                                nc2 = hr * ws[m]
                                src = ps[64 * m : 64 * m + 64, :].rearrange(
                                    "p (two x) -> p two x", two=2
                                )[:, :, 0:nc2]
                                dst = g[
                                    64 * m : 64 * m + 64,
                                    nwidth : nwidth + 2 * nc2,
                                ].rearrange("p (two n) -> p two n", two=2)
                                if m == 0:
                                    nc.vector.tensor_copy(dst, src)
                                else:
                                    nc.scalar.copy(dst, src)
                    # Ship this group of the batch.
                    if is_edge:
                        for pair in pairs:
                            bi, pj = divmod(pair, NPJ)
                            hr = _nr(bi) // 2
                            side = "L" if pj == 0 else "R"
                            for m in range(2):
                                dram, w = edram[f"gout{side}{m}"]
                                n2 = 2 * hr * w
                                eo = _EDGE_OFF[w][0][bi]
                                nc.sync.dma_start(
                                    dram[:, eo : eo + n2],
                                    g[64 * m : 64 * m + 64,
                                      nwidth : nwidth + n2],
                                )
                    elif pairs:
                        nc.sync.dma_start(
                            gout[:, nout : nout + nwidth], g[:, 0:nwidth]
                        )
                        nout += nwidth
    nc.compile()
    _NC_CACHE[key] = nc
    return nc


def _shard_inputs(x1, x2):
    """Per-core inputs: core k -> batch k//2, row-half k%2.

    half=1 slabs are flipped vertically (both operands), which maps the
    correlation to the same computation with u and i reversed; the zero
    row-halo then sits at padded rows [0,10) for every core, and only
    padded rows [10,84) ship.
    """
    in_maps = []
    for k in range(N_CORES):
        b, half = k // 2, k % 2
        if half == 0:
            x1s = x1[b, :, 0:ROWS, :]
            x2s = x2[b, :, 0:X2R, :]  # padded rows 10:84 = global 0:74
        else:
            x1s = x1[b, :, ROWS:H, :][:, ::-1, :]
            # flipped padded rows 10:84 = global rows 127..54 descending
            x2s = x2[b, :, H - X2R : H, :][:, ::-1, :]
        x1s = np.ascontiguousarray(
            x1s.reshape(C, NBI, DI, NBJ, DJ)
            .transpose(0, 1, 3, 2, 4)
            .reshape(C, NBLK, DI * DJ)
        ).astype(np.float16)
        x2s = np.ascontiguousarray(x2s).astype(ml_dtypes.float8_e3m4)
        in_maps.append({"x1": x1s, "x2": x2s})
    return in_maps


# Band-extraction index arrays (built once).  Within a block, partition
# p = il*DJ + jl; a full 28x28 window stores free f = (il+u)*NS + (jl+v).
_IL = np.arange(DI).reshape(DI, 1, 1, 1)
_JL = np.arange(DJ).reshape(1, DJ, 1, 1)
_U = np.arange(WIN).reshape(1, 1, WIN, 1)
_V = np.arange(WIN).reshape(1, 1, 1, WIN)


def _band(arr, nr, w, rshift, cshift):
    """Band-extract blocks whose stored window is row/col-trimmed.

    arr: [..., 64, nr, w]; stored row = il+u+rshift, col = jl+v+cshift;
    out of range means the output is exactly 0 (zero-pad region).
    Returns [..., DI, DJ, WIN, WIN].
    """
    ro = _IL + _U + rshift
    co = _JL + _V + cshift
    valid = (ro >= 0) & (ro < nr) & (co >= 0) & (co < w)
    part = _IL * DJ + _JL
    band = arr[..., part, np.clip(ro, 0, nr - 1), np.clip(co, 0, w - 1)]
    return np.where(valid, band, np.float32(0.0))


def kernel(x1: np.ndarray, x2: np.ndarray) -> np.ndarray:
    x1 = np.asarray(x1, dtype=np.float32)
    x2 = np.asarray(x2, dtype=np.float32)
    nc = _build_nc()
    in_maps = _shard_inputs(x1, x2)
    # Retry once: a freshly-claimed device occasionally reports a transient
    # NRT_EXEC_UNIT_UNRECOVERABLE on the first execution.
    try:
        res = run_bass_kernel_spmd(nc, in_maps, core_ids=list(range(N_CORES)))
    except Exception:
        import time as _time

        _time.sleep(5.0)
        res = run_bass_kernel_spmd(nc, in_maps, core_ids=list(range(N_CORES)))
    out = np.empty((B, WIN * WIN, H, W), dtype=np.float32)
    corr = np.empty((WIN, WIN, ROWS, W), dtype=np.float32)
    for k in range(N_CORES):
        b, half = k // 2, k % 2
        r = res.results[k]
        gnorm = r["gout"].astype(np.float32)
        for pair in range(NPAIR):
            bi, pj = divmod(pair, NPJ)
            nr = _nr(bi)
            rshift = DI * bi - _tr0(bi)  # -10 / -2 / 0
            if pair in V4SET:
                wU = _wu(pj)
                base = _NORM_OFF[pair]
                arr2 = gnorm[:, base : base + 2 * (nr // 2) * wU].reshape(
                    2, 64, nr, wU
                )
                for m in range(2):
                    cs = m * DJ - (PAD if pj == 0 else 0)
                    band = _band(arr2[m], nr, wU, rshift, cs)
                    corr[:, :, bi * DI : (bi + 1) * DI,
                         (pj * 2 + m) * DJ : (pj * 2 + m + 1) * DJ] = (
                        band.transpose(2, 3, 0, 1)
                    )
            elif pj in (0, NPJ - 1):
                side = "L" if pj == 0 else "R"
                for m in range(2):
                    w = _EDGE_W[pj][m]
                    cs = (m * DJ - PAD) if pj == 0 else 0
                    eo = _EDGE_OFF[w][0][bi]
                    arr = (
                        r[f"gout{side}{m}"][:, eo : eo + nr * w]
                        .reshape(64, nr, w)
                        .astype(np.float32)
                    )
                    band = _band(arr, nr, w, rshift, cs)
                    corr[:, :, bi * DI : (bi + 1) * DI,
                         (pj * 2 + m) * DJ : (pj * 2 + m + 1) * DJ] = (
                        band.transpose(2, 3, 0, 1)
                    )
            else:
                base = _NORM_OFF[pair]
                arr2 = gnorm[:, base : base + 2 * (nr // 2) * NS].reshape(
                    2, 64, nr, NS
                )
                for m in range(2):
                    band = _band(arr2[m], nr, NS, rshift, 0)
                    corr[:, :, bi * DI : (bi + 1) * DI,
                         (pj * 2 + m) * DJ : (pj * 2 + m + 1) * DJ] = (
                        band.transpose(2, 3, 0, 1)
                    )
        if half == 0:
            out[b, :, 0:ROWS, :] = corr.reshape(WIN * WIN, ROWS, W)
        else:
            out[b, :, ROWS:H, :] = corr[::-1, :, ::-1, :].reshape(
                WIN * WIN, ROWS, W
            )
    return out


# revision 49
# speedup vs baseline: 1.0574x; 1.0102x over previous
"""Trainium2 Bass kernel for the FlowNet-style correlation module.

out[b, u*21+v, i, j] = sum_c x1[b,c,i,j] * x2pad[b,c,i+u,j+v]
with x1, x2: [4, 128, 128, 128] fp32, pad=10, window 21x21 (441 output channels).

Strategy
--------
Sharding: 8 cores = (batch 4) x (H halves). Each core handles one batch's
64-row slab: x1 slice [C=128, 64, 128] (host-rearranged into 8x8 pixel
blocks) and an x2 slice with halos. The half=1 cores' slabs are FLIPPED
vertically on the host (corr with both operands i-flipped equals the
original with u and i reversed, undone during extraction), so every core's
zero row-halo sits uniformly at padded rows [0,10). Neither the +-10 row
halo nor the +-10 col halo is ever materialized: windows that would straddle
a halo stream only their valid rows/cols, and the Gram entries that would
multiply the zero pad are neither computed nor shipped (the host extraction
knows those outputs are exactly 0).

Per core the correlation is computed as blocked Gram matmuls on the tensor
engine: each 8x8 pixel block of x1 (M=64) is a stationary operand on one
64-column half of the PE array (tile_position=(0,64m)), two blocks per PSUM
tile, each streaming its own (up to) 28x28 x2pad window split into two
row-halves that sit in separate PSUM banks of a 2-bank tile.

Inputs are fp16 single-pass (error budget 2e-2 vs measured ~5e-4 end-to-end;
fp8 variants measured over budget). Gram tiles are copied PSUM->SBUF with
fp32->fp16 conversion (DVE/ACT alternating) and shipped fp16. The j-edge
blocks (window width 18 or 26 instead of 28) get per-member 64-partition
copies and partition-ranged output DMAs into dedicated edge tensors.

Each output pixel's 21x21 window is a per-partition band of its Gram tile; a
per-partition-offset band cannot be expressed by any on-chip access pattern
(and DMA has no PSUM route), so the device ships the full Gram tiles and the
host extracts the band while unsharding. 8x8 blocks trade a little Gram
inflation (784/441 untrimmed vs 672/441 for 4x8) for half the tensor-engine
streaming charge (2 x 784 vs 4 x 672 columns per 128 pixels), which is what
binds once the output ships as fp16.

Per-core traffic: 4.5MB in + 11.5MB Gram out (fp16), ~44.5us of DMA at
~360GB/s, overlapped with a ~40us serial PE span; TimelineSim makespan
48.9us (baseline 93.9us).
"""

import ml_dtypes
import numpy as np

import concourse.mybir as mybir
import concourse.tile as tile
from concourse import bacc
from concourse.bass_utils import run_bass_kernel_spmd

# Problem constants (hardcoded; kernel.py must be self-contained).
B, C, H, W = 4, 128, 128, 128
PAD = 10
WIN = 21  # correlation window side; WIN**2 = 441 output channels
N_CORES = 8
ROWS = H // 2  # 64 output rows per core
X2R = ROWS + PAD  # 74 shipped x2 rows (padded rows [10, 84))

# Pixel blocking: 8x8 blocks (M=64), two blocks per PSUM tile via PE
# column-tiling at tile_position (0,0)/(0,64).
DI, DJ = 8, 8
NR, NS = DI + WIN - 1, DJ + WIN - 1  # 28, 28
NBI, NBJ = ROWS // DI, W // DJ  # 8, 16
NBLK = NBI * NBJ  # 128
NPJ = NBJ // 2  # 8 pairs of j-adjacent blocks per block-row
NPAIR = NBLK // 2  # 64

F32 = mybir.dt.float32
F16 = mybir.dt.float16
F8E3 = mybir.dt.float8e3


def _wj(j0):
    """Valid window col width for a block whose padded window starts at j0."""
    return min(j0 + NS, PAD + W) - max(j0, PAD)


def _tr0(bi):
    """First valid padded window row for block-row bi."""
    return max(DI * bi, PAD)


def _nr(bi):
    """Valid window row count for block-row bi (18 / 26 / 28)."""
    return DI * bi + NR - _tr0(bi)


# Per-m col widths of the two edge pair columns (pj=0 and pj=7).
WL0, WL1 = _wj(0), _wj(DJ)  # 18, 26
WR0, WR1 = _wj(W - 2 * DJ), _wj(W - DJ)  # 26, 18
_EDGE_W = {0: (WL0, WL1), NPJ - 1: (WR0, WR1)}

# Pairs computed as one M=128 union-window matmul pair (both blocks as a
# single stationary, streaming the 36-col union): costs ~28% more shipped
# bytes but ~36% less serial PE time; applied to the tail where the PE is
# the critical path. Their (former) edge members ship full-width.
V4SET = frozenset({23, 24, 26, 27, 31, 32, 33, 35, 36, 37})


def _wu(pj):
    # Union window width for a V4 pair (both members).
    lo = max(pj * 2 * DJ, PAD)
    hi = min(pj * 2 * DJ + DJ + NS, PAD + W)
    return hi - lo


_NC_CACHE = {}

# Tunables (overridable via _build_nc kwargs for experiments).
GRAM_BUFS = 8
PSUM_BUFS = 4  # [128,1024] fp32 tiles = 2 banks each; 4 bufs = all 8 banks
DVE_MOD = 2  # normal pair copied by DVE iff pair % DVE_MOD == 0, else ACT
# Output DMA schedule: entries are either a pair count (contiguous) or an
# explicit pair list; must cover all NPAIR pairs. Each batch may contain at
# most one edge pair (pj 0 or 7). The tail ends with the lone edge pair so
# the post-copy drain is two small partition-ranged transfers.
QSCHED = [4] * 16
# Input chunk schedule: list of ("x1", blk_lo, blk_hi) / ("x2", row_lo, row_hi)
# (x2 rows in shipped [0,74) coords) in issue order. First chunks are small
# so the PE starts early.
IN_SCHED = [
    ("x1", 0, 16),
    ("x2", 0, 9),
    ("x2", 9, 18),
    ("x2", 18, 34),
    ("x1", 16, 64),
    ("x2", 34, 58),
    ("x1", 64, 128),
    ("x2", 58, 74),
]
# PE clock-ramp warmup: x1-only matmuls into a scratch PSUM tile while the
# first x2 chunk is still in flight (the p-state model charges matmuls 2x
# until the engine has been busy 3us).
WARMUPS = 6


def _batches(qsched):
    """qsched -> list of (normal_pairs, edge_pairs) per DMA batch."""
    batches = []
    q0 = 0
    seen = []
    for qb in qsched:
        pairs = qb if isinstance(qb, list) else list(range(q0, q0 + qb))
        q0 = pairs[-1] + 1 if isinstance(qb, list) else q0 + qb
        seen += pairs
        normals = [p for p in pairs
                   if p % NPJ not in (0, NPJ - 1) or p in V4SET]
        edges = [p for p in pairs
                 if p % NPJ in (0, NPJ - 1) and p not in V4SET]
        assert len(edges) <= 1, "at most one edge pair per DMA batch"
        batches.append((normals, edges))
    assert sorted(seen) == list(range(NPAIR))
    return batches


# Shipping layout for the DEFAULT schedule (extraction must match kernel()).
def _pair_width(pair):
    bi, pj = divmod(pair, NPJ)
    w = _wu(pj) if pair in V4SET else NS
    return 2 * (_nr(bi) // 2) * w


_NORM_OFF = {}
_off = 0
for _ns_, _es_ in _batches(QSCHED):
    for _p in _ns_:
        _NORM_OFF[_p] = _off
        _off += _pair_width(_p)
GOUT_COLS = _off
# Edge tensors are indexed [64, sum over bi of 2*hr(bi)*w] in bi order.
_EDGE_OFF = {}
for _w in (WL0, WL1, WR0, WR1):
    offs, o = [], 0
    for _bi in range(NBI):
        offs.append(o)
        o += 2 * (_nr(_bi) // 2) * _w
    _EDGE_OFF[_w] = (offs, o)


def _build_nc(
    qsched=None, in_sched=None, gram_bufs=None, psum_bufs=None, dve_mod=None,
    warmups=None,
):
    gram_bufs = GRAM_BUFS if gram_bufs is None else gram_bufs
    psum_bufs = PSUM_BUFS if psum_bufs is None else psum_bufs
    qsched = list(QSCHED) if qsched is None else list(qsched)
    in_sched = list(IN_SCHED) if in_sched is None else list(in_sched)
    dve_mod = DVE_MOD if dve_mod is None else dve_mod
    warmups = WARMUPS if warmups is None else warmups
    key = (
        tuple(tuple(q) if isinstance(q, list) else q for q in qsched),
        tuple(map(tuple, in_sched)), gram_bufs, psum_bufs, dve_mod, warmups,
    )
    if key in _NC_CACHE:
        return _NC_CACHE[key]
    nc = bacc.Bacc("TRN2", target_bir_lowering=False, debug=False, num_devices=N_CORES)
    # x1 arrives host-rearranged so each 8x8 block's 64 pixels are contiguous
    # (the matmul stationary operand AP must have a single free dimension).
    x1d = nc.dram_tensor("x1", [C, NBLK, DI * DJ], F16, kind="ExternalInput")
    # Only valid rows/cols ship (contiguous SBUF destination keeps the
    # transfer above the 512B full-bandwidth knee).
    x2d = nc.dram_tensor("x2", [C, X2R, W], F8E3, kind="ExternalInput")
    # Normal pairs (pj 1..6), in shipping order (see _NORM_OFF).
    gout = nc.dram_tensor("gout", [128, GOUT_COLS], F16, kind="ExternalOutput")
    # Edge blocks: partition-ranged tensors, one per block column position.
    edram = {
        "goutL0": (nc.dram_tensor("goutL0", [64, _EDGE_OFF[WL0][1]], F16,
                                  kind="ExternalOutput"), WL0),
        "goutL1": (nc.dram_tensor("goutL1", [64, _EDGE_OFF[WL1][1]], F16,
                                  kind="ExternalOutput"), WL1),
        "goutR0": (nc.dram_tensor("goutR0", [64, _EDGE_OFF[WR0][1]], F16,
                                  kind="ExternalOutput"), WR0),
        "goutR1": (nc.dram_tensor("goutR1", [64, _EDGE_OFF[WR1][1]], F16,
                                  kind="ExternalOutput"), WR1),
    }

    batches = _batches(qsched)

    with tile.TileContext(nc) as tc:
        with (
            tc.tile_pool(name="inp", bufs=1) as inp,
            tc.tile_pool(name="gram", bufs=gram_bufs) as gp,
            tc.tile_pool(name="psum", bufs=psum_bufs, space="PSUM") as pp,
        ):
            x1t = inp.tile([C, NBLK, DI * DJ], F16)
            x2t = inp.tile([C, X2R, W], F8E3)
            wsrc = None
            if warmups:
                # PE clock warmup source: a tile that only the Pool memset
                # writes, so the warmup matmuls depend on nothing else and
                # ramp the engine to full clock before the first real
                # operand lands. The warmups write into the FIRST pair's
                # PSUM tile (emitted in the pair loop below): a separate
                # scratch PSUM tile would overcommit the 8-bank PSUM pool.
                wsrc = inp.tile([C, 8, DI * DJ], F16)
                nc.gpsimd.memset(wsrc[:], 1)
            for entry in in_sched:
                kind, lo, hi = entry[:3]
                if kind == "x1":
                    nc.sync.dma_start(x1t[:, lo:hi, :], x1d[:, lo:hi, :])
                else:
                    nc.sync.dma_start(x2t[:, lo:hi, :], x2d[:, lo:hi, :])

            nout = 0  # running col offset into gout (normal pairs)
            for normals, edges in batches:
                nwidth = sum(_pair_width(p) for p in normals)
                ewide = 0
                if edges:
                    ebi = edges[0] // NPJ
                    ewide = 2 * (_nr(ebi) // 2) * max(WL1, WR0)
                g = gp.tile([128, nwidth + ewide], F16, tag="g")
                for pairs, is_edge in ((normals, False), (edges, True)):
                    goff = 0
                    for pair in pairs:
                        bi, pj = divmod(pair, NPJ)
                        hr = _nr(bi) // 2
                        rb = _tr0(bi) - PAD  # x2t row of the window start
                        ps = pp.tile([128, 1024], F32, tag="ps")
                        if wsrc is not None:
                            # Warmups go into this first tile's banks; the
                            # real matmuls below reset (start=True) every
                            # region the copy later reads.
                            for _ in range(warmups):
                                nc.tensor.matmul(
                                    ps[0:64, 0:512], wsrc[:, 0, :], wsrc[:],
                                    start=True, stop=True,
                                    tile_position=(0, 0),
                                    skip_group_check=True,
                                )
                            wsrc = None
                        if pair in V4SET:
                            # One M=128 matmul per row-half: both blocks as
                            # a single stationary streaming the union window.
                            wU = _wu(pj)
                            clo = max(pj * 2 * DJ - PAD, 0)
                            blk = bi * NBJ + pj * 2
                            for h in range(2):
                                rlo = rb + hr * h
                                nc.tensor.matmul(
                                    ps[:, 512 * h : 512 * h + hr * wU],
                                    x1t[:, blk : blk + 2, :],
                                    x2t[:, rlo : rlo + hr, clo : clo + wU],
                                    start=True, stop=True,
                                    tile_position=(0, 0),
                                    skip_group_check=True,
                                )
                            wp = 2 * hr * wU
                            src = ps.rearrange("p (two x) -> p two x", two=2)[
                                :, :, 0 : hr * wU
                            ]
                            dst = g[:, goff : goff + wp].rearrange(
                                "p (two n) -> p two n", two=2
                            )
                            if pair % dve_mod == 0:
                                nc.vector.tensor_copy(dst, src)
                            else:
                                nc.scalar.copy(dst, src)
                            goff += wp
                            continue
                        ws = [_wj((pj * 2 + m) * DJ) for m in range(2)]
                        # h-outer: both members' h=0 matmuls need only the
                        # first x2 row-chunk, so the PE starts sooner while
                        # the next chunk is still in flight.
                        for h in range(2):
                            for m in range(2):
                                j0 = (pj * 2 + m) * DJ
                                w = ws[m]
                                clo = max(j0 - PAD, 0)
                                blk = bi * NBJ + pj * 2 + m
                                rlo = rb + hr * h
                                nc.tensor.matmul(
                                    ps[64 * m : 64 * m + 64,
                                       512 * h : 512 * h + hr * w],
                                    x1t[:, blk, :],
                                    x2t[:, rlo : rlo + hr, clo : clo + w],
                                    start=True, stop=True,
                                    tile_position=(0, 64 * m),
                                    skip_group_check=True,
                                )
                        if not is_edge:
                            # Strided fp32->fp16 copies spanning both banks.
                            wp = 2 * hr * NS
                            src = ps.rearrange("p (two x) -> p two x", two=2)[
                                :, :, 0 : hr * NS
                            ]
                            dst = g[:, goff : goff + wp].rearrange(
                                "p (two n) -> p two n", two=2
                            )
                            use_dve = pair % dve_mod == 0
                            if pair in (61, 62):
                                # Tail swap: the drain-gating pair 62 goes to
                                # the faster ACT copy (61 takes DVE instead).
                                use_dve = not use_dve
                            if dve_mod == 0:
                                # Split each copy: one bank half per engine
                                # (halves the per-pair copy latency).
                                nc.vector.tensor_copy(dst[:, 0], src[:, 0])
                                nc.scalar.copy(dst[:, 1], src[:, 1])
                            elif use_dve:
                                nc.vector.tensor_copy(dst, src)
                            else:
                                nc.scalar.copy(dst, src)
                            goff += wp
                        else:
                            # Per-member 64-partition copies (widths differ).
                            for m in range(2):
                                nc2 = hr * ws[m]
                                src = ps[64 * m : 64 * m + 64, :].rearrange(
                                    "p (two x) -> p two x", two=2
                                )[:, :, 0:nc2]
                                dst = g[
                                    64 * m : 64 * m + 64,
                                    nwidth : nwidth + 2 * nc2,
                                ].rearrange("p (two n) -> p two n", two=2)
                                if m == 0:
                                    nc.vector.tensor_copy(dst, src)
                                else:
                                    nc.scalar.copy(dst, src)
                    # Ship this group of the batch.
                    if is_edge:
                        for pair in pairs:
                            bi, pj = divmod(pair, NPJ)
                            hr = _nr(bi) // 2
                            side = "L" if pj == 0 else "R"
                            for m in range(2):
                                dram, w = edram[f"gout{side}{m}"]
                                n2 = 2 * hr * w
                                eo = _EDGE_OFF[w][0][bi]
                                nc.sync.dma_start(
                                    dram[:, eo : eo + n2],
                                    g[64 * m : 64 * m + 64,
                                      nwidth : nwidth + n2],
                                )
                    elif pairs:
                        nc.sync.dma_start(
                            gout[:, nout : nout + nwidth], g[:, 0:nwidth]
                        )
                        nout += nwidth
    nc.compile()
    _NC_CACHE[key] = nc
    return nc


def _shard_inputs(x1, x2):
    """Per-core inputs: core k -> batch k//2, row-half k%2.

    half=1 slabs are flipped vertically (both operands), which maps the
    correlation to the same computation with u and i reversed; the zero
    row-halo then sits at padded rows [0,10) for every core, and only
    padded rows [10,84) ship.
    """
    in_maps = []
    for k in range(N_CORES):
        b, half = k // 2, k % 2
        if half == 0:
            x1s = x1[b, :, 0:ROWS, :]
            x2s = x2[b, :, 0:X2R, :]  # padded rows 10:84 = global 0:74
        else:
            x1s = x1[b, :, ROWS:H, :][:, ::-1, :]
            # flipped padded rows 10:84 = global rows 127..54 descending
            x2s = x2[b, :, H - X2R : H, :][:, ::-1, :]
        x1s = np.ascontiguousarray(
            x1s.reshape(C, NBI, DI, NBJ, DJ)
            .transpose(0, 1, 3, 2, 4)
            .reshape(C, NBLK, DI * DJ)
        ).astype(np.float16)
        x2s = np.ascontiguousarray(x2s).astype(ml_dtypes.float8_e3m4)
        in_maps.append({"x1": x1s, "x2": x2s})
    return in_maps


# Band-extraction index arrays (built once).  Within a block, partition
# p = il*DJ + jl; a full 28x28 window stores free f = (il+u)*NS + (jl+v).
_IL = np.arange(DI).reshape(DI, 1, 1, 1)
_JL = np.arange(DJ).reshape(1, DJ, 1, 1)
_U = np.arange(WIN).reshape(1, 1, WIN, 1)
_V = np.arange(WIN).reshape(1, 1, 1, WIN)


def _band(arr, nr, w, rshift, cshift):
    """Band-extract blocks whose stored window is row/col-trimmed.

    arr: [..., 64, nr, w]; stored row = il+u+rshift, col = jl+v+cshift;
    out of range means the output is exactly 0 (zero-pad region).
    Returns [..., DI, DJ, WIN, WIN].
    """
    ro = _IL + _U + rshift
    co = _JL + _V + cshift
    valid = (ro >= 0) & (ro < nr) & (co >= 0) & (co < w)
    part = _IL * DJ + _JL
    band = arr[..., part, np.clip(ro, 0, nr - 1), np.clip(co, 0, w - 1)]
    return np.where(valid, band, np.float32(0.0))


def kernel(x1: np.ndarray, x2: np.ndarray) -> np.ndarray:
    x1 = np.asarray(x1, dtype=np.float32)
    x2 = np.asarray(x2, dtype=np.float32)
    nc = _build_nc()
    in_maps = _shard_inputs(x1, x2)
    # Retry once: a freshly-claimed device occasionally reports a transient
    # NRT_EXEC_UNIT_UNRECOVERABLE on the first execution.
    try:
        res = run_bass_kernel_spmd(nc, in_maps, core_ids=list(range(N_CORES)))
    except Exception:
        import time as _time

        _time.sleep(5.0)
        res = run_bass_kernel_spmd(nc, in_maps, core_ids=list(range(N_CORES)))
    out = np.empty((B, WIN * WIN, H, W), dtype=np.float32)
    corr = np.empty((WIN, WIN, ROWS, W), dtype=np.float32)
    for k in range(N_CORES):
        b, half = k // 2, k % 2
        r = res.results[k]
        gnorm = r["gout"].astype(np.float32)
        for pair in range(NPAIR):
            bi, pj = divmod(pair, NPJ)
            nr = _nr(bi)
            rshift = DI * bi - _tr0(bi)  # -10 / -2 / 0
            if pair in V4SET:
                wU = _wu(pj)
                base = _NORM_OFF[pair]
                arr2 = gnorm[:, base : base + 2 * (nr // 2) * wU].reshape(
                    2, 64, nr, wU
                )
                for m in range(2):
                    cs = m * DJ - (PAD if pj == 0 else 0)
                    band = _band(arr2[m], nr, wU, rshift, cs)
                    corr[:, :, bi * DI : (bi + 1) * DI,
                         (pj * 2 + m) * DJ : (pj * 2 + m + 1) * DJ] = (
                        band.transpose(2, 3, 0, 1)
                    )
            elif pj in (0, NPJ - 1):
                side = "L" if pj == 0 else "R"
                for m in range(2):
                    w = _EDGE_W[pj][m]
                    cs = (m * DJ - PAD) if pj == 0 else 0
                    eo = _EDGE_OFF[w][0][bi]
                    arr = (
                        r[f"gout{side}{m}"][:, eo : eo + nr * w]
                        .reshape(64, nr, w)
                        .astype(np.float32)
                    )
                    band = _band(arr, nr, w, rshift, cs)
                    corr[:, :, bi * DI : (bi + 1) * DI,
                         (pj * 2 + m) * DJ : (pj * 2 + m + 1) * DJ] = (
                        band.transpose(2, 3, 0, 1)
                    )
            else:
                base = _NORM_OFF[pair]
                arr2 = gnorm[:, base : base + 2 * (nr // 2) * NS].reshape(
                    2, 64, nr, NS
                )
                for m in range(2):
                    band = _band(arr2[m], nr, NS, rshift, 0)
                    corr[:, :, bi * DI : (bi + 1) * DI,
                         (pj * 2 + m) * DJ : (pj * 2 + m + 1) * DJ] = (
                        band.transpose(2, 3, 0, 1)
                    )
        if half == 0:
            out[b, :, 0:ROWS, :] = corr.reshape(WIN * WIN, ROWS, W)
        else:
            out[b, :, ROWS:H, :] = corr[::-1, :, ::-1, :].reshape(
                WIN * WIN, ROWS, W
            )
    return out
